# revision 54
# baseline (speedup 1.0000x reference)
"""Trainium2 Bass kernel for 16-head causal MHA (B=2, T=2048, D=1024, fp32 I/O).

Sharding: core c owns batch c//4 and head-quad c%4 (heads 4q..4q+3, as two
head-pairs). It computes Q/K/V projections for its 256 q/k/v dims, causal
attention for its 4 heads, and a partial output [2048, 1024] (bf16); the host
sums the 4 partials per batch in f64. One batch per core halves the partial-
output drain (PSUM->SBUF casts + DMA) and the x^T input DMA vs 2-batch cores.

Per-core device program, matmul inputs bf16 (2 cols/PE-cycle + fast weight
load), fp32 PSUM accumulation. The PE matmul stream is the bottleneck
(~137us busy of ~158us wall); the schedule keeps it dense from the first
projection to the last output block:
  - Q^T, K^T = W.T @ x^T (weights stationary, N=512 moving blocks)
  - V natural = x @ Wv with a ones column per head (denominator for free);
    one V tile holds all 4 heads so each chunk drains in one strided copy
  - attention in S^T layout per head-pair: the two heads' K=64 contractions
    sit at PE base partitions 0/64 (row-tiled, concurrent); exp on ScalarE
    with the 1/sqrt(dk) scale folded in; causality = skipping fully-masked
    blocks + one [128,1024] staircase multiply per diagonal 2-chunk group
    (mask2 packs the 4 staircase patterns contiguously)
  - normalization: denominator rows drain to 32-aligned partitions of a
    [97,512] tile per qn (memset to 1.0 once so junk rows stay finite),
    1/den = exp(-ln(den)) per pair in two ScalarE [33,512] calls to bf16,
    then ONE bf16 K=33 selection matmul per pair broadcasts the two rcp
    rows across the heads' partitions (replaces the old per-he K=1 fp32
    matmuls, ~20us of PE) and one in-place multiply normalizes the pair
  - partial out per 128-query chunk: two accumulating K=128 matmuls (one
    per head-pair), cast to bf16, DMA'd per 1024-col row block; the last
    qn casts alternate ScalarE/DVE and DMA per 512-col half
  - diagonal-block S/ctx matmuls and exp trim their moving width to the
    causally-live queries (the masked region is never computed)
  - x^T and the weights are host-pre-blocked so every DMA reads contiguous
    multi-KB runs, spread over the sync/scalar/gpsimd trigger queues in
    consumption order (the input phase is aggregate-HBM-bound at
    ~358GB/s); x^T arrives per 512-token block so attention starts early
  - attention-phase filler (later projections, out-proj, norm) is metered
    by estimated PE-time per group so the ScalarE exp stream never starves
    behind a filler burst, with deadline pressure from the next qn's needs;
    the base budget is deliberately below the average filler demand
    (750ns vs ~1.1us) so filler defers toward its deadline and the next
    S tile enters the PE queue right as exp frees its PSUM slot
    (950->750 measured ~1.5us; 600 showed no further gain)
  - the PE is never allowed to idle >1us: dependency-free dummy matmuls
    bridge DMA jitter in the prologue and the norm chain in the endgame,
    because an idle PE triggers a HAM clock downshift to 4/8 duty that
    roughly doubles matmul time until well after the bubble ends.

Infrastructure: the external walrus allows only ONE sync wait per
instruction; a post-pass hoists extra waits onto single-wait no-ops and the
TileContext closing drain is split into a chain of single-wait drains. The
closing drain also skips the device-side semaphore/dma reset + second
barrier (~7us after the last DMA; the NEFF executes once per kernel()
call). Known-bad variants (do not retry): GpSimd cannot touch PSUM (den
copies must stay on DVE/ScalarE); custom-DVE ops (reciprocal_approx_fast)
fail this walrus ("ISA wrong length"); GpSimd tensor_mul for the staircase
masks is too slow and stalls the exp->ctx chain; reusing the freed ps_ctx
banks for the endgame out-proj accumulators corrupts the output. Note
~±2-30us run-to-run device clock variance on these cores - compare
schedules by the min of several runs.
"""

import numpy as np

import bass_rust
from bass_rust import ScopedClock
import concourse.bass as bass
import concourse.mybir as mybir
import concourse.tile as tile

F32 = mybir.dt.float32
BF16 = mybir.dt.bfloat16
F32R = BF16
B, T, D = 2, 2048, 1024
NCORES = 8
P = 128          # partitions / feature chunk
FC = D // P      # 8 feature chunks
QW = 512         # query block width (PSUM bank)
QN = T // QW     # 4 query blocks
KC = T // P      # 16 key chunks
DK = 64
CW = 256         # q/k/v dims per core (4 heads x 64)
NPAIR = 2        # head-pairs per core

# Set True to offload half the diagonal-mask multiplies to GpSimd.
GPSIMD_MASKS = False

# ---------------------------------------------------------------------------
# TileContext drain fix: the external walrus in this container allows only ONE
# sync wait per instruction, but Tile's closing drain packs one wait per active
# proc. Split it into a chain of single-wait drains (same semantics).
_PATCHED = False


def _patched_drain_and_barrier(self, tick_clock, wait_clock):
    nc = self.nc
    drain_inst = nc.sync.drain()
    wait_clock.add_sem_waits(
        drain_inst.ins, ScopedClock({None: tick_clock.global_clock})
    )
    si = drain_inst.ins.sync_info
    waits = list(si.on_wait) if si is not None else []
    if len(waits) > 1:
        si.on_wait = [waits[0]]
        drain_inst.ins.sync_info = si
        for w in waits[1:]:
            d2 = nc.sync.drain()
            si2 = d2.ins.sync_info
            if si2 is None:
                si2 = bass_rust.SyncInfo(on_wait=[w], on_update=[])
            else:
                si2.on_wait = [w]
            d2.ins.sync_info = si2
    nc.all_engine_barrier()
    assert self.sems is not None
    popped = nc._tile_sem_poison_stack.pop()
    assert popped is self._sem_poison
    # End of program: skip the device-side semaphore/dma-queue reset and the
    # second barrier (the trace shows that cascade costs ~7us after the last
    # DMA completes). The NEFF executes once per kernel() call, so nothing
    # re-reads the dirty semaphores.


def _apply_tile_patch():
    global _PATCHED
    if not _PATCHED:
        tile.TileContext._drain_and_barrier = _patched_drain_and_barrier
        _PATCHED = True


def _split_multi_waits(nc):
    """Post-pass: the external walrus accepts only 1 sync wait per
    instruction (2 for EventSemaphore). Tile emits more. Hoist extra waits
    onto same-engine no-ops inserted just before. For compute engines this
    is identical semantics (the engine blocks either way). For DMA triggers
    it turns queue-side waits into SP-side blocking, which is safe in this
    forward-dataflow single-block program (every wait's producer precedes
    the trigger in the scheduled stream); CoreSim re-validates no-deadlock."""
    for f in nc.m.functions:
        for bb in f.blocks:
            new = []
            for ins in bb.instructions:
                si = ins.sync_info
                if si is not None:
                    cap = 2 if isinstance(ins, mybir.InstEventSemaphore) else 1
                    waits = list(si.on_wait)
                    if len(waits) > cap:
                        for w in waits[:-cap]:
                            nop = mybir.InstNoOp(
                                name=nc.get_next_instruction_name(),
                                engine=ins.engine,
                                sync_info=bass_rust.SyncInfo(
                                    on_wait=[w], on_update=[]
                                ),
                                bass_nofuse=True,
                            )
                            nc.register_instruction(nop, overwrite=True)
                            new.append(nop)
                        si.on_wait = waits[-cap:]
                        ins.sync_info = si
                new.append(ins)
            bb.instructions = new


# ---------------------------------------------------------------------------
_PROGRAM = None


def build_program():
    global _PROGRAM
    if _PROGRAM is not None:
        return _PROGRAM
    _apply_tile_patch()
    Exp = mybir.ActivationFunctionType.Exp
    Log = mybir.ActivationFunctionType.Ln
    Copy = mybir.ActivationFunctionType.Copy

    nc = bass.Bass()
    xt_d = nc.declare_dram_parameter("xt", [P, QN * FC * QW], F32R, isOutput=False)
    wq_d = nc.declare_dram_parameter("wq", [P, NPAIR * FC * P], F32R, isOutput=False)
    wk_d = nc.declare_dram_parameter("wk", [P, NPAIR * FC * P], F32R, isOutput=False)
    wv_d = nc.declare_dram_parameter("wv", [P, FC * CW], F32R, isOutput=False)
    wo_d = nc.declare_dram_parameter("wo", [CW, D], F32R, isOutput=False)
    mask2_d = nc.declare_dram_parameter("mask2", [P, 4 * QW], F32R, isOutput=False)
    sel_d = nc.declare_dram_parameter("sel", [33, P], F32R, isOutput=False)
    out_d = nc.declare_dram_parameter("out", [T, D], F32R, isOutput=True)

    with tile.TileContext(nc) as tc:
        from contextlib import ExitStack

        ctx = ExitStack()
        with ctx:
            consts = ctx.enter_context(tc.tile_pool(name="consts", bufs=1))
            xt_pool = ctx.enter_context(tc.tile_pool(name="xt", bufs=1))
            qk_pool = ctx.enter_context(tc.tile_pool(name="qk", bufs=1))
            v_pool = ctx.enter_context(tc.tile_pool(name="v", bufs=1))
            exp_pool = ctx.enter_context(tc.tile_pool(name="exp", bufs=8))
            ctxt_pool = ctx.enter_context(tc.tile_pool(name="ctxt", bufs=1))
            rcp_pool = ctx.enter_context(tc.tile_pool(name="rcp", bufs=2))
            ob_pool = ctx.enter_context(tc.tile_pool(name="ob", bufs=3))

            ps_s = ctx.enter_context(tc.tile_pool(name="ps_s", bufs=2, space="PSUM"))
            ps_ctx = ctx.enter_context(
                tc.tile_pool(name="ps_ctx", bufs=1, space="PSUM")
            )
            ps_px = ctx.enter_context(tc.tile_pool(name="ps_px", bufs=2, space="PSUM"))

            # ---- constants ----
            wq_sb = consts.tile([P, NPAIR, FC, P], F32R, tag="wq")
            wk_sb = consts.tile([P, NPAIR, FC, P], F32R, tag="wk")
            wv_sb = consts.tile([P, FC, CW], F32R, tag="wv")
            wo_sb = [
                consts.tile([P, D], F32R, tag=f"wo{p}", name=f"wo_sb{p}")
                for p in range(NPAIR)
            ]
            mask2_sb = consts.tile([P, 4 * QW], F32R, tag="mask2")
            sel_sb = consts.tile([33, P], F32R, tag="sel")
            # DMA triggers spread across engine queues so the ~0.6us
            # per-trigger cost parallelizes and x^T lands ASAP
            # HAM warm-up: ~5us of dummy matmuls on a memset tile flip the
            # PE clock gate to 8/8 during the input-DMA window, so the real
            # projection prologue runs at 2.4 GHz instead of 1.2
            warm_sb = consts.tile([P, QW], F32R, tag="warm")
            nc.vector.memset(warm_sb, 0.0)
            for wi in range(12):
                wps = ps_px.tile([P, QW], F32, tag="px", name=f"warm{wi}")
                nc.tensor.matmul(
                    wps, lhsT=warm_sb[:, 0:P], rhs=warm_sb, start=True, stop=True
                )

            # x^T arrives by query/key block: qn0's attention needs only
            # tokens 0-511. All inputs are host-pre-blocked so every DMA
            # reads contiguous multi-KB runs per partition at full rate.
            # Queue plan orders each trigger queue by when data is needed;
            # xt block 1 rides the otherwise-idle vector queue so it lands
            # before qn1's attention (~22us) instead of behind wv.
            # one tile per 512-token block: precise DMA->matmul dependencies
            # so attention on early blocks never waits on later blocks' DMAs
            xt_rc = [
                xt_pool.tile([P, FC, QW], F32R, tag=f"xt{rc}", name=f"xt_rc{rc}")
                for rc in range(QN)
            ]

            def xt_blk(rc, fclo, fchi):
                return xt_d[
                    :, rc * FC * QW + fclo * QW : rc * FC * QW + fchi * QW
                ].rearrange("p (f c) -> p f c", c=QW)

            def w_blk(w_d, pair):
                return w_d[
                    :, pair * FC * P : (pair + 1) * FC * P
                ].rearrange("p (f c) -> p f c", c=P)

            # queue plan (all three trigger queues contend for ~358GB/s of
            # HBM, per-queue throughput is arbitration-dependent): spread the
            # early-needed tensors across the queues in consumption order so
            # no single queue's crawl stalls the projection prologue
            nc.sync.dma_start(out=wq_sb[:, 0], in_=w_blk(wq_d, 0))
            nc.scalar.dma_start(out=xt_rc[0][:, 0:4, :], in_=xt_blk(0, 0, 4))
            nc.gpsimd.dma_start(out=wk_sb[:, 0], in_=w_blk(wk_d, 0))
            nc.scalar.dma_start(out=wq_sb[:, 1], in_=w_blk(wq_d, 1))
            nc.gpsimd.dma_start(out=xt_rc[0][:, 4:FC, :], in_=xt_blk(0, 4, FC))
            nc.scalar.dma_start(out=wk_sb[:, 1], in_=w_blk(wk_d, 1))
            nc.sync.dma_start(
                out=wv_sb, in_=wv_d.rearrange("p (f c) -> p f c", c=CW)
            )
            nc.scalar.dma_start(out=xt_rc[1][:, 0:4, :], in_=xt_blk(1, 0, 4))
            nc.scalar.dma_start(out=xt_rc[1][:, 4:FC, :], in_=xt_blk(1, 4, FC))
            nc.sync.dma_start(out=mask2_sb, in_=mask2_d[:, :])
            nc.scalar.dma_start(out=sel_sb, in_=sel_d[:, :])
            nc.gpsimd.dma_start(out=xt_rc[2][:, 0:4, :], in_=xt_blk(2, 0, 4))
            nc.gpsimd.dma_start(out=xt_rc[2][:, 4:FC, :], in_=xt_blk(2, 4, FC))
            nc.gpsimd.dma_start(out=xt_rc[3], in_=xt_blk(3, 0, FC))
            for p in range(NPAIR):
                nc.sync.dma_start(out=wo_sb[p], in_=wo_d[p * P : (p + 1) * P, :])

            qt = [
                qk_pool.tile([P, T], F32R, tag=f"qt{p}", name=f"qt{p}")
                for p in range(NPAIR)
            ]
            kt = [
                qk_pool.tile([P, T], F32R, tag=f"kt{p}", name=f"kt{p}")
                for p in range(NPAIR)
            ]
            # one V tile for all 4 heads: group g=2*pair+he at cols [g*65,
            # g*65+65) per kc (64 v-dims + the ones/denominator column)
            v_sb = v_pool.tile([P, KC, 4 * 65], F32R, tag="v", name="v_sb")
            ctxt = [
                ctxt_pool.tile([P, T], F32R, tag=f"c{p}", name=f"ctxt{p}")
                for p in range(NPAIR)
            ]

            def _qk_half(cell, rc, pair, which, lo, hi):
                w_sb, dst = (wq_sb, qt[pair]) if which == 0 else (wk_sb, kt[pair])
                if lo == 0:
                    cell["ps"] = ps_px.tile(
                        [P, QW], F32, tag="px", name=f"qk{rc}{pair}{which}"
                    )
                ps = cell["ps"]
                for fc in range(lo, hi):
                    nc.tensor.matmul(
                        ps,
                        lhsT=w_sb[:, pair, fc, :],
                        rhs=xt_rc[rc][:, fc, :],
                        start=(fc == 0),
                        stop=(fc == FC - 1),
                    )
                if hi == FC:
                    nc.vector.tensor_copy(dst[:, rc * QW : (rc + 1) * QW], ps)

            def emit_qk_one(rc, pair, which):
                cell = {}
                _qk_half(cell, rc, pair, which, 0, 4)
                _qk_half(cell, rc, pair, which, 4, FC)

            def enq_qk(rc, pair, which):
                cell = {}
                projq.append(
                    (None, lambda: _qk_half(cell, rc, pair, which, 0, 4), 852)
                )
                projq.append(
                    (
                        ("qk", rc, pair, which),
                        lambda: _qk_half(cell, rc, pair, which, 4, FC),
                        852,
                    )
                )

            def _v_half(cell, kc, lo, hi):
                rc, ko = divmod(kc, 4)
                if lo == 0:
                    cell["ps"] = ps_px.tile([P, CW], F32, tag="px", name=f"vps{kc}")
                ps = cell["ps"]
                for fc in range(lo, hi):
                    nc.tensor.matmul(
                        ps,
                        lhsT=xt_rc[rc][:, fc, ko * P : (ko + 1) * P],
                        rhs=wv_sb[:, fc, :],
                        start=(fc == 0),
                        stop=(fc == FC - 1),
                    )
                if hi == FC:
                    # all 4 heads' 64 cols in one strided copy (g-step 65)
                    nc.vector.tensor_copy(
                        v_sb[:, kc, :]
                        .rearrange("p (g x) -> p g x", g=4)[:, :, 0:DK],
                        ps.rearrange("p (g x) -> p g x", g=4),
                    )

            def emit_v_kc(kc):
                cell = {}
                _v_half(cell, kc, 0, 4)
                _v_half(cell, kc, 4, FC)

            def enq_v(kc):
                cell = {}
                projq.append((None, lambda: _v_half(cell, kc, 0, 4), 428))
                projq.append((("v", kc), lambda: _v_half(cell, kc, 4, FC), 428))

            def emit_ones_cols():
                # memset (no DMA dependency, unlike sourcing from mask2 —
                # a mask2 wait here would head-block the whole DVE queue)
                for g in range(4):
                    nc.vector.memset(
                        v_sb[:, :, g * 65 + DK : g * 65 + DK + 1], 1.0
                    )

            def emit_drain_pair(qn, pair, ctx_ps, den97, scalar_den=False):
                # drain unnormalized ctx to bf16 SBUF and the denominator
                # rows to 32-aligned partitions of the shared den tile, so
                # the PSUM accumulators free up immediately. For the final
                # pair the den copies ride ScalarE (idle by then) so they
                # overlap the DVE ctx casts on the endgame critical path.
                for he in range(2):
                    nc.vector.tensor_copy(
                        ctxt[pair][
                            he * DK : (he + 1) * DK, qn * QW : (qn + 1) * QW
                        ],
                        ctx_ps[he][0:DK, :],
                    )
                    r = 32 * (2 * pair + he)
                    dstd = den97[r : r + 1, :]
                    srcd = ctx_ps[he][DK : DK + 1, :]
                    if scalar_den:
                        nc.scalar.activation(out=dstd, in_=srcd, func=Copy)
                    else:
                        nc.vector.tensor_copy(dstd, srcd)

            rcp_tiles = {}

            def emit_norm_rcp_pair(qn, pair, den97):
                # 1/den for the pair's two heads (rows 0/32), bf16 output.
                # Junk rows between them stay finite (den buffers are memset
                # to 1.0 once) and the selection matmul zeroes their
                # contribution. Mid-kernel the reciprocal runs on the idle
                # GpSimd so the ScalarE exp stream isn't interrupted; the
                # latency-critical last qn uses ScalarE ln/exp (exp is done).
                rcp33 = rcp_pool.tile(
                    [33, QW], F32R, tag=f"rcp{pair}", name=f"rcp{qn}{pair}"
                )
                den33 = den97[64 * pair : 64 * pair + 33, :]
                lnd = rcp_pool.tile(
                    [33, QW], F32, tag=f"lnd{pair}", name=f"lnd{qn}{pair}"
                )
                nc.scalar.activation(out=lnd, in_=den33, func=Log)
                nc.scalar.activation(out=rcp33, in_=lnd, func=Exp, scale=-1.0)
                rcp_tiles[(qn, pair)] = rcp33

            def emit_norm_mul_pair(qn, pair):
                # bf16 selection matmul broadcasts rcp rows 0/32 across the
                # two heads' 64 partitions each (~0.3us vs ~2.7us for the old
                # per-he K=1 fp32 matmuls), then one in-place multiply for
                # the whole pair tile
                rcp33 = rcp_tiles.pop((qn, pair))
                bc = ps_px.tile([P, QW], F32, tag="px", name=f"bc{qn}{pair}")
                nc.tensor.matmul(bc, lhsT=sel_sb, rhs=rcp33, start=True, stop=True)
                dst = ctxt[pair][:, qn * QW : (qn + 1) * QW]
                nc.vector.tensor_mul(dst, dst, bc)

            def _outproj_half(cell, qn, i2, c2):
                rc = qn * 4 + i2
                if c2 == 0:
                    cell["ob"] = ob_pool.tile([P, D], F32R, tag="ob", name=f"ob{rc}")
                ob = cell["ob"]
                ps = ps_px.tile([P, QW], F32, tag="px", name=f"o{rc}{c2}")
                for pair in range(NPAIR):
                    nc.tensor.matmul(
                        ps,
                        lhsT=ctxt[pair][:, rc * P : (rc + 1) * P],
                        rhs=wo_sb[pair][:, c2 * QW : (c2 + 1) * QW],
                        start=(pair == 0),
                        stop=(pair == NPAIR - 1),
                    )
                dstc = ob[:, c2 * QW : (c2 + 1) * QW]
                if qn == QN - 1:
                    # endgame: alternate casts across ScalarE/DVE and DMA
                    # each 512-col half immediately so the final drain is
                    # short
                    if c2 == 0:
                        nc.scalar.activation(out=dstc, in_=ps, func=Copy)
                    else:
                        nc.vector.tensor_copy(dstc, ps)
                    # alternate the 8 final half-block DMAs across the sync
                    # and scalar trigger queues so the last blocks drain in
                    # parallel instead of serializing on one queue
                    dma_eng = nc.sync if (i2 + c2) % 2 == 0 else nc.scalar
                    dma_eng.dma_start(
                        out=out_d[rc * P : (rc + 1) * P, c2 * QW : (c2 + 1) * QW],
                        in_=dstc,
                    )
                else:
                    nc.vector.tensor_copy(dstc, ps)
                    if c2 == 1:
                        nc.sync.dma_start(
                            out=out_d[rc * P : (rc + 1) * P, :], in_=ob
                        )

            def enq_outproj(qn, i2):
                cell = {}
                slackq.append((lambda: _outproj_half(cell, qn, i2, 0), 426))
                slackq.append((lambda: _outproj_half(cell, qn, i2, 1), 426))

            # ---- flat software-pipelined attention stream ----
            # The ScalarE exp stream is the bottleneck; S matmuls run one
            # group ahead of the ctx matmuls so exp(g+1) never waits on PE
            # work that is queued behind ctx(g). Projections for qn+1, the
            # normalization, and the output projection are interleaved as
            # "filler" slices between attention groups so the PE/DVE queues
            # stay dense (HAM-warm) without starving the exp pipeline.
            from collections import deque

            projq = deque()   # QK/V projection slices: gate later attention
            slackq = deque()  # norm + out-proj slices: no downstream deadline
            emitted = set()

            def pop_one():
                # returns the PE-time estimate (ns) of the popped slice
                if projq:
                    key, fn, cost = projq.popleft()
                    fn()
                    emitted.add(key)
                    return cost
                if slackq:
                    fn, cost = slackq.popleft()
                    fn()
                    return cost
                return 0

            def pops(rem_groups):
                # meter filler by estimated PE time per attention group: the
                # ScalarE exp stream paces a group at ~2.1us of which S+ctx
                # take ~1.3us of PE; overshooting filler in one group starves
                # the exp stream (it waits on the next S behind the filler).
                # Deadline pressure: this qn's queued projections must land
                # before the next qn's attention starts.
                pcost = sum(e[2] for e in projq)
                budget = max(750, pcost // max(rem_groups, 1))
                spent = pops_carry[0]
                n = 0
                while (projq or slackq) and spent < budget and n < 8:
                    spent += pop_one()
                    n += 1
                pops_carry[0] = max(0, spent - budget)

            pops_carry = [0]

            def need(*keys):
                # drain proj fillers until all producer keys are emitted:
                # Tile derives dependencies from trace order, so a consumer
                # must never be traced before its producer
                for k in keys:
                    while k not in emitted:
                        key, fn, cost = projq.popleft()
                        fn()
                        emitted.add(key)

            # prologue: fc0-3 of all four QK projections first (they only
            # need the first xt half), then fc4-7 (second xt half), then V
            # kc0/1 (wv arrives last) — ordered by DMA arrival so the
            # in-order PE queue never head-blocks on a later transfer
            def jitter_dummies(n, label):
                # dependency-free matmuls between DMA-gated segments: absorb
                # input-DMA jitter without idling the PE (an idle PE triggers
                # a HAM downshift to 4/8 duty that outlasts the bubble)
                for wi in range(n):
                    wps = ps_s.tile(
                        [P, 2 * QW], F32, tag="s", name=f"jd_{label}_{wi}"
                    )
                    nc.tensor.matmul(
                        wps[:, 0:QW],
                        lhsT=warm_sb[:, 0:P],
                        rhs=warm_sb,
                        start=True,
                        stop=True,
                    )

            qk_cells = {}
            for pair in range(2):
                for w in range(2):
                    qk_cells[(pair, w)] = {}
                    _qk_half(qk_cells[(pair, w)], 0, pair, w, 0, 4)
                    jitter_dummies(2, f"p{pair}{w}")
            # second xt half lands ~3us after the fc0-3 work drains (the
            # input phase is aggregate-HBM-bound); bridge with dummies so
            # the clock never downshifts
            jitter_dummies(8, "xt0b")
            for pair in range(2):
                for w in range(2):
                    _qk_half(qk_cells[(pair, w)], 0, pair, w, 4, FC)
                    emitted.add(("qk", 0, pair, w))
            for kc in range(2):
                emit_v_kc(kc)
                emitted.add(("v", kc))
            emit_ones_cols()
            # den buffers start finite (1.0) so Reciprocal of junk rows
            # can't produce NaN that the selection matmul would 0*NaN
            for i in range(2):
                nc.vector.memset(
                    rcp_pool.tile([97, QW], F32, tag="den", name=f"den_init{i}"),
                    1.0,
                )
            for kc in (2, 3):
                enq_v(kc)

            flat = []
            for qn in range(QN):
                for pair in range(NPAIR):
                    nkc = 4 * (qn + 1)
                    for kc2 in range(0, nkc, 2):
                        flat.append((qn, pair, kc2, nkc))

            s_tiles = {}

            def emit_S(i):
                qn, pair, kc2, nkc = flat[i]
                need(*[("qk", r, pair, w) for r in range(qn + 1) for w in range(2)])
                sp = {
                    he: ps_s.tile([P, 2 * QW], F32, tag="s", name=f"s{i}{he}")
                    for he in range(2)
                }
                s_tiles[i] = sp
                # he0/he1 interleaved: the K=64 matmuls land on PE row groups
                # 0-63 / 64-127 back-to-back (concurrent row tiling).
                # Diagonal chunks only produce scores for queries >= 128j
                # (the rest is causally masked), so trim the moving width.
                for half in range(2):
                    kc = kc2 + half
                    qoff = max(0, (kc - 4 * qn) * P)
                    for he in range(2):
                        nc.tensor.matmul(
                            sp[he][:, half * QW + qoff : (half + 1) * QW],
                            lhsT=kt[pair][
                                he * DK : (he + 1) * DK, kc * P : (kc + 1) * P
                            ],
                            rhs=qt[pair][
                                he * DK : (he + 1) * DK,
                                qn * QW + qoff : (qn + 1) * QW,
                            ],
                            start=True,
                            stop=True,
                        )

            ctx_tiles = {}
            den_tiles = {}
            mask_alt = [0]
            emit_S(0)
            for i, (qn, pair, kc2, nkc) in enumerate(flat):
                if kc2 == 0:
                    ctx_tiles[(qn, pair)] = {
                        he: ps_ctx.tile(
                            [65, QW], F32, tag=f"x{he}", name=f"ctx{qn}{pair}{he}"
                        )
                        for he in range(2)
                    }
                    if pair == 0:
                        den_tiles[qn] = rcp_pool.tile(
                            [97, QW], F32, tag="den", name=f"den{qn}"
                        )
                        if qn + 1 < QN:
                            for p2 in range(NPAIR):
                                for w in range(2):
                                    enq_qk(qn + 1, p2, w)
                            for kc in range(4 * (qn + 1), 4 * (qn + 1) + 4):
                                enq_v(kc)
                ctx_ps = ctx_tiles[(qn, pair)]
                sp = s_tiles.pop(i)
                dg = 0 if kc2 == 4 * qn else (1 if kc2 == 4 * qn + 2 else -1)
                goff = max(0, (kc2 - 4 * qn) * P)  # group exp start col
                es = {}
                for he in range(2):
                    e = exp_pool.tile([P, 2 * QW], F32R, tag="exp", name=f"e{i}{he}")
                    nc.scalar.activation(
                        out=e[:, goff:], in_=sp[he][:, goff:], func=Exp, scale=0.125
                    )
                    if dg >= 0:
                        eng = nc.vector
                        if GPSIMD_MASKS:
                            mask_alt[0] ^= 1
                            if mask_alt[0]:
                                eng = nc.gpsimd
                        eng.tensor_mul(
                            e[:, goff:],
                            e[:, goff:],
                            mask2_sb[:, dg * 2 * QW + goff : (dg + 1) * 2 * QW],
                        )
                    es[he] = e
                if i + 1 < len(flat):
                    emit_S(i + 1)
                need(("v", kc2), ("v", kc2 + 1))
                for half in range(2):
                    kc = kc2 + half
                    qoff = max(0, (kc - 4 * qn) * P)
                    for he in range(2):
                        g = 2 * pair + he
                        nc.tensor.matmul(
                            ctx_ps[he][:, qoff:],
                            lhsT=v_sb[:, kc, g * 65 : g * 65 + 65],
                            rhs=es[he][:, half * QW + qoff : (half + 1) * QW],
                            start=(kc == 0),
                            stop=(kc == nkc - 1),
                        )
                if kc2 == nkc - 2:
                    last = qn == QN - 1 and pair == NPAIR - 1
                    emit_drain_pair(
                        qn, pair, ctx_ps, den_tiles[qn], scalar_den=last
                    )
                    del ctx_tiles[(qn, pair)]
                    d97 = den_tiles[qn]
                    slackq.append(
                        (
                            lambda qn=qn, p=pair, d=d97: emit_norm_rcp_pair(
                                qn, p, d
                            ),
                            0,
                        )
                    )
                    slackq.append(
                        (lambda qn=qn, p=pair: emit_norm_mul_pair(qn, p), 213)
                    )
                    if pair == NPAIR - 1:
                        for i2 in range(4):
                            enq_outproj(qn, i2)
                pops(nkc - (pair * (nkc // 2) + kc2 // 2))
            # endgame HAM keep-warm: the final norm chain leaves the PE
            # briefly idle, which would downshift the clock to 4/8 duty for
            # the whole output projection; dependency-free dummy matmuls
            # bridge the gap (ps_s banks are free after the last exp)
            for wi in range(23):
                wps = ps_s.tile([P, 2 * QW], F32, tag="s", name=f"ewarm{wi}")
                nc.tensor.matmul(
                    wps[:, 0:QW],
                    lhsT=warm_sb[:, 0:P],
                    rhs=warm_sb,
                    start=True,
                    stop=True,
                )
            while projq or slackq:
                pop_one()

    _split_multi_waits(nc)
    _PROGRAM = nc
    return nc


def _make_mask2():
    # mask2[:, 512j:512j+512] = stair(j): [k, q] = 1.0 iff q >= 128j + k
    k = np.arange(P)[:, None]
    q = np.arange(QW)[None, :]
    blocks = [(q >= 128 * j + k).astype(np.float32) for j in range(4)]
    return np.concatenate(blocks, axis=1)


def make_in_maps(x, Wq, Wk, Wv, Wo):
    import ml_dtypes

    nd = ml_dtypes.bfloat16 if F32R == BF16 else np.float32
    x = np.asarray(x, dtype=np.float32)
    mask2 = _make_mask2().astype(nd)
    # selection matrix for the softmax-normalization broadcast matmul:
    # bc[m] = rcp[0] for m<64 (he0), rcp[32] for m>=64 (he1)
    sel = np.zeros((33, P), np.float32)
    sel[0, 0:DK] = 1.0
    sel[32, DK:P] = 1.0
    sel = sel.astype(nd)
    Wq, Wk, Wv, Wo = (np.asarray(w, dtype=np.float32) for w in (Wq, Wk, Wv, Wo))
    # blocked x^T: [p, rc, f, c] so device rc-slices are contiguous
    xts = [
        np.ascontiguousarray(
            x[b].reshape(QN, QW, FC, P).transpose(3, 0, 2, 1).reshape(P, -1)
        ).astype(nd)
        for b in range(B)
    ]
    def qk_block(W, cols):
        # [p][pair][f][c]: per-pair slices are contiguous 2KB/partition
        s = W[:, cols].reshape(FC, P, NPAIR, P)
        return np.ascontiguousarray(
            s.transpose(1, 2, 0, 3).reshape(P, -1)
        ).astype(nd)

    def v_block(W, cols):
        # [p][f][c]: one contiguous 4KB run per partition
        s = W[:, cols].reshape(FC, P, CW)
        return np.ascontiguousarray(s.transpose(1, 0, 2).reshape(P, -1)).astype(nd)

    in_maps = []
    for c in range(NCORES):
        b, q4 = divmod(c, NCORES // B)
        cols = slice(q4 * CW, (q4 + 1) * CW)
        in_maps.append(
            {
                "xt": xts[b],
                "wq": qk_block(Wq, cols),
                "wk": qk_block(Wk, cols),
                "wv": v_block(Wv, cols),
                "wo": np.ascontiguousarray(Wo[cols, :]).astype(nd),
                "mask2": mask2,
                "sel": sel,
            }
        )
    return in_maps


def reduce_outputs(results):
    """Sum the per-core bf16 partials (4 cores per batch) in f64."""
    out = np.zeros((B, T, D), dtype=np.float64)
    for c in range(NCORES):
        b = c // (NCORES // B)
        out[b] += np.asarray(results[c]["out"], dtype=np.float64)
    return out.astype(np.float32)


def kernel(x, Wq, Wk, Wv, Wo):
    from concourse.bass_utils import run_bass_kernel_spmd

    nc = build_program()
    in_maps = make_in_maps(x, Wq, Wk, Wv, Wo)
    res = run_bass_kernel_spmd(nc, in_maps, core_ids=list(range(NCORES)))
    return reduce_outputs(res.results)


if __name__ == "__main__":
    rng = np.random.default_rng(0)
    s = 1.0 / np.sqrt(D)
    ins = {
        "x": rng.standard_normal((B, T, D)).astype(np.float32),
        "Wq": (rng.standard_normal((D, D)) * s).astype(np.float32),
        "Wk": (rng.standard_normal((D, D)) * s).astype(np.float32),
        "Wv": (rng.standard_normal((D, D)) * s).astype(np.float32),
        "Wo": (rng.standard_normal((D, D)) * (1.0 / np.sqrt(D))).astype(np.float32),
    }
    out = kernel(**ins)
    print("out", out.shape, out.dtype, float(np.abs(out).max()))



# revision 55
# speedup vs baseline: 1.0068x; 1.0068x over previous
"""Trainium2 Bass kernel for 16-head causal MHA (B=2, T=2048, D=1024, fp32 I/O).

Sharding: core c owns batch c//4 and head-quad c%4 (heads 4q..4q+3, as two
head-pairs). It computes Q/K/V projections for its 256 q/k/v dims, causal
attention for its 4 heads, and a partial output [2048, 1024] (bf16); the host
sums the 4 partials per batch in f64. One batch per core halves the partial-
output drain (PSUM->SBUF casts + DMA) and the x^T input DMA vs 2-batch cores.

Per-core device program, matmul inputs bf16 (2 cols/PE-cycle + fast weight
load), fp32 PSUM accumulation. The PE matmul stream is the bottleneck
(~137us busy of ~158us wall); the schedule keeps it dense from the first
projection to the last output block:
  - Q^T, K^T = W.T @ x^T (weights stationary, N=512 moving blocks)
  - V natural = x @ Wv with a ones column per head (denominator for free);
    one V tile holds all 4 heads so each chunk drains in one strided copy
  - attention in S^T layout per head-pair: the two heads' K=64 contractions
    sit at PE base partitions 0/64 (row-tiled, concurrent); exp on ScalarE
    with the 1/sqrt(dk) scale folded in; causality = skipping fully-masked
    blocks + one [128,1024] staircase multiply per diagonal 2-chunk group
    (mask2 packs the 4 staircase patterns contiguously)
  - normalization: denominator rows drain to 32-aligned partitions of a
    [97,512] tile per qn (memset to 1.0 once so junk rows stay finite),
    1/den = exp(-ln(den)) per pair in two ScalarE [33,512] calls to bf16,
    then ONE bf16 K=33 selection matmul per pair broadcasts the two rcp
    rows across the heads' partitions (replaces the old per-he K=1 fp32
    matmuls, ~20us of PE) and one in-place multiply normalizes the pair
  - partial out per 128-query chunk: two accumulating K=128 matmuls (one
    per head-pair), cast to bf16, DMA'd per 1024-col row block; the last
    qn casts alternate ScalarE/DVE and DMA per 512-col half
  - diagonal-block S/ctx matmuls and exp trim their moving width to the
    causally-live queries (the masked region is never computed)
  - x^T and the weights are host-pre-blocked so every DMA reads contiguous
    multi-KB runs, spread over the sync/scalar/gpsimd trigger queues in
    consumption order (the input phase is aggregate-HBM-bound at
    ~358GB/s); x^T arrives per 512-token block so attention starts early
  - attention-phase filler (later projections, out-proj, norm) is metered
    by estimated PE-time per group so the ScalarE exp stream never starves
    behind a filler burst, with deadline pressure from the next qn's needs;
    the base budget is deliberately below the average filler demand
    (750ns vs ~1.1us) so filler defers toward its deadline and the next
    S tile enters the PE queue right as exp frees its PSUM slot
    (950->750 measured ~1.5us; 600 showed no further gain)
  - the PE is never allowed to idle >1us: dependency-free dummy matmuls
    bridge DMA jitter in the prologue and the norm chain in the endgame,
    because an idle PE triggers a HAM clock downshift to 4/8 duty that
    roughly doubles matmul time until well after the bubble ends.

Infrastructure: the external walrus allows only ONE sync wait per
instruction; a post-pass hoists extra waits onto single-wait no-ops and the
TileContext closing drain is split into a chain of single-wait drains. The
closing drain also skips the device-side semaphore/dma reset + second
barrier (~7us after the last DMA; the NEFF executes once per kernel()
call). Known-bad variants (do not retry): GpSimd cannot touch PSUM (den
copies must stay on DVE/ScalarE); custom-DVE ops (reciprocal_approx_fast)
fail this walrus ("ISA wrong length"); GpSimd tensor_mul for the staircase
masks is too slow and stalls the exp->ctx chain; reusing the freed ps_ctx
banks for the endgame out-proj accumulators corrupts the output. Note
~±2-30us run-to-run device clock variance on these cores - compare
schedules by the min of several runs.
"""

import numpy as np

import bass_rust
from bass_rust import ScopedClock
import concourse.bass as bass
import concourse.mybir as mybir
import concourse.tile as tile

F32 = mybir.dt.float32
BF16 = mybir.dt.bfloat16
F32R = BF16
B, T, D = 2, 2048, 1024
NCORES = 8
P = 128          # partitions / feature chunk
FC = D // P      # 8 feature chunks
QW = 512         # query block width (PSUM bank)
QN = T // QW     # 4 query blocks
KC = T // P      # 16 key chunks
DK = 64
CW = 256         # q/k/v dims per core (4 heads x 64)
NPAIR = 2        # head-pairs per core

# Set True to offload half the diagonal-mask multiplies to GpSimd.
GPSIMD_MASKS = False

# ---------------------------------------------------------------------------
# TileContext drain fix: the external walrus in this container allows only ONE
# sync wait per instruction, but Tile's closing drain packs one wait per active
# proc. Split it into a chain of single-wait drains (same semantics).
_PATCHED = False


def _patched_drain_and_barrier(self, tick_clock, wait_clock):
    nc = self.nc
    drain_inst = nc.sync.drain()
    wait_clock.add_sem_waits(
        drain_inst.ins, ScopedClock({None: tick_clock.global_clock})
    )
    si = drain_inst.ins.sync_info
    waits = list(si.on_wait) if si is not None else []
    if len(waits) > 1:
        si.on_wait = [waits[0]]
        drain_inst.ins.sync_info = si
        for w in waits[1:]:
            d2 = nc.sync.drain()
            si2 = d2.ins.sync_info
            if si2 is None:
                si2 = bass_rust.SyncInfo(on_wait=[w], on_update=[])
            else:
                si2.on_wait = [w]
            d2.ins.sync_info = si2
    nc.all_engine_barrier()
    assert self.sems is not None
    popped = nc._tile_sem_poison_stack.pop()
    assert popped is self._sem_poison
    # End of program: skip the device-side semaphore/dma-queue reset and the
    # second barrier (the trace shows that cascade costs ~7us after the last
    # DMA completes). The NEFF executes once per kernel() call, so nothing
    # re-reads the dirty semaphores.


def _apply_tile_patch():
    global _PATCHED
    if not _PATCHED:
        tile.TileContext._drain_and_barrier = _patched_drain_and_barrier
        _PATCHED = True


def _split_multi_waits(nc):
    """Post-pass: the external walrus accepts only 1 sync wait per
    instruction (2 for EventSemaphore). Tile emits more. Hoist extra waits
    onto same-engine no-ops inserted just before. For compute engines this
    is identical semantics (the engine blocks either way). For DMA triggers
    it turns queue-side waits into SP-side blocking, which is safe in this
    forward-dataflow single-block program (every wait's producer precedes
    the trigger in the scheduled stream); CoreSim re-validates no-deadlock."""
    for f in nc.m.functions:
        for bb in f.blocks:
            new = []
            for ins in bb.instructions:
                si = ins.sync_info
                if si is not None:
                    cap = 2 if isinstance(ins, mybir.InstEventSemaphore) else 1
                    waits = list(si.on_wait)
                    if len(waits) > cap:
                        for w in waits[:-cap]:
                            nop = mybir.InstNoOp(
                                name=nc.get_next_instruction_name(),
                                engine=ins.engine,
                                sync_info=bass_rust.SyncInfo(
                                    on_wait=[w], on_update=[]
                                ),
                                bass_nofuse=True,
                            )
                            nc.register_instruction(nop, overwrite=True)
                            new.append(nop)
                        si.on_wait = waits[-cap:]
                        ins.sync_info = si
                new.append(ins)
            bb.instructions = new


# ---------------------------------------------------------------------------
_PROGRAM = None


def build_program():
    global _PROGRAM
    if _PROGRAM is not None:
        return _PROGRAM
    _apply_tile_patch()
    Exp = mybir.ActivationFunctionType.Exp
    Log = mybir.ActivationFunctionType.Ln
    Copy = mybir.ActivationFunctionType.Copy

    nc = bass.Bass()
    xt_d = nc.declare_dram_parameter("xt", [P, QN * FC * QW], F32R, isOutput=False)
    wq_d = nc.declare_dram_parameter("wq", [P, NPAIR * FC * P], F32R, isOutput=False)
    wk_d = nc.declare_dram_parameter("wk", [P, NPAIR * FC * P], F32R, isOutput=False)
    wv_d = nc.declare_dram_parameter("wv", [P, FC * CW], F32R, isOutput=False)
    wo_d = nc.declare_dram_parameter("wo", [CW, D], F32R, isOutput=False)
    mask2_d = nc.declare_dram_parameter("mask2", [P, 4 * QW], F32R, isOutput=False)
    sel_d = nc.declare_dram_parameter("sel", [33, P], F32R, isOutput=False)
    out_d = nc.declare_dram_parameter("out", [T, D], F32R, isOutput=True)

    with tile.TileContext(nc) as tc:
        from contextlib import ExitStack

        ctx = ExitStack()
        with ctx:
            consts = ctx.enter_context(tc.tile_pool(name="consts", bufs=1))
            xt_pool = ctx.enter_context(tc.tile_pool(name="xt", bufs=1))
            qk_pool = ctx.enter_context(tc.tile_pool(name="qk", bufs=1))
            v_pool = ctx.enter_context(tc.tile_pool(name="v", bufs=1))
            exp_pool = ctx.enter_context(tc.tile_pool(name="exp", bufs=8))
            ctxt_pool = ctx.enter_context(tc.tile_pool(name="ctxt", bufs=1))
            rcp_pool = ctx.enter_context(tc.tile_pool(name="rcp", bufs=2))
            ob_pool = ctx.enter_context(tc.tile_pool(name="ob", bufs=3))

            ps_s = ctx.enter_context(tc.tile_pool(name="ps_s", bufs=2, space="PSUM"))
            ps_ctx = ctx.enter_context(
                tc.tile_pool(name="ps_ctx", bufs=1, space="PSUM")
            )
            ps_px = ctx.enter_context(tc.tile_pool(name="ps_px", bufs=2, space="PSUM"))

            # ---- constants ----
            wq_sb = consts.tile([P, NPAIR, FC, P], F32R, tag="wq")
            wk_sb = consts.tile([P, NPAIR, FC, P], F32R, tag="wk")
            wv_sb = consts.tile([P, FC, CW], F32R, tag="wv")
            wo_sb = [
                consts.tile([P, D], F32R, tag=f"wo{p}", name=f"wo_sb{p}")
                for p in range(NPAIR)
            ]
            mask2_sb = consts.tile([P, 4 * QW], F32R, tag="mask2")
            sel_sb = consts.tile([33, P], F32R, tag="sel")
            # DMA triggers spread across engine queues so the ~0.6us
            # per-trigger cost parallelizes and x^T lands ASAP
            # HAM warm-up: ~5us of dummy matmuls on a memset tile flip the
            # PE clock gate to 8/8 during the input-DMA window, so the real
            # projection prologue runs at 2.4 GHz instead of 1.2
            warm_sb = consts.tile([P, QW], F32R, tag="warm")
            nc.vector.memset(warm_sb, 0.0)
            for wi in range(12):
                wps = ps_px.tile([P, QW], F32, tag="px", name=f"warm{wi}")
                nc.tensor.matmul(
                    wps, lhsT=warm_sb[:, 0:P], rhs=warm_sb, start=True, stop=True
                )

            # x^T arrives by query/key block: qn0's attention needs only
            # tokens 0-511. All inputs are host-pre-blocked so every DMA
            # reads contiguous multi-KB runs per partition at full rate.
            # Queue plan orders each trigger queue by when data is needed;
            # xt block 1 rides the otherwise-idle vector queue so it lands
            # before qn1's attention (~22us) instead of behind wv.
            # one tile per 512-token block: precise DMA->matmul dependencies
            # so attention on early blocks never waits on later blocks' DMAs
            xt_rc = [
                xt_pool.tile([P, FC, QW], F32R, tag=f"xt{rc}", name=f"xt_rc{rc}")
                for rc in range(QN)
            ]

            def xt_blk(rc, fclo, fchi):
                return xt_d[
                    :, rc * FC * QW + fclo * QW : rc * FC * QW + fchi * QW
                ].rearrange("p (f c) -> p f c", c=QW)

            def w_blk(w_d, pair):
                return w_d[
                    :, pair * FC * P : (pair + 1) * FC * P
                ].rearrange("p (f c) -> p f c", c=P)

            # queue plan (all three trigger queues contend for ~358GB/s of
            # HBM, per-queue throughput is arbitration-dependent): spread the
            # early-needed tensors across the queues in consumption order so
            # no single queue's crawl stalls the projection prologue
            nc.sync.dma_start(out=wq_sb[:, 0], in_=w_blk(wq_d, 0))
            nc.scalar.dma_start(out=xt_rc[0][:, 0:4, :], in_=xt_blk(0, 0, 4))
            nc.gpsimd.dma_start(out=wk_sb[:, 0], in_=w_blk(wk_d, 0))
            nc.scalar.dma_start(out=wq_sb[:, 1], in_=w_blk(wq_d, 1))
            nc.gpsimd.dma_start(out=xt_rc[0][:, 4:FC, :], in_=xt_blk(0, 4, FC))
            nc.scalar.dma_start(out=wk_sb[:, 1], in_=w_blk(wk_d, 1))
            nc.sync.dma_start(
                out=wv_sb, in_=wv_d.rearrange("p (f c) -> p f c", c=CW)
            )
            nc.scalar.dma_start(out=xt_rc[1][:, 0:4, :], in_=xt_blk(1, 0, 4))
            nc.scalar.dma_start(out=xt_rc[1][:, 4:FC, :], in_=xt_blk(1, 4, FC))
            nc.sync.dma_start(out=mask2_sb, in_=mask2_d[:, :])
            nc.scalar.dma_start(out=sel_sb, in_=sel_d[:, :])
            nc.gpsimd.dma_start(out=xt_rc[2][:, 0:4, :], in_=xt_blk(2, 0, 4))
            nc.gpsimd.dma_start(out=xt_rc[2][:, 4:FC, :], in_=xt_blk(2, 4, FC))
            nc.gpsimd.dma_start(out=xt_rc[3], in_=xt_blk(3, 0, FC))
            for p in range(NPAIR):
                nc.sync.dma_start(out=wo_sb[p], in_=wo_d[p * P : (p + 1) * P, :])

            qt = [
                qk_pool.tile([P, T], F32R, tag=f"qt{p}", name=f"qt{p}")
                for p in range(NPAIR)
            ]
            kt = [
                qk_pool.tile([P, T], F32R, tag=f"kt{p}", name=f"kt{p}")
                for p in range(NPAIR)
            ]
            # one V tile for all 4 heads: group g=2*pair+he at cols [g*65,
            # g*65+65) per kc (64 v-dims + the ones/denominator column)
            v_sb = v_pool.tile([P, KC, 4 * 65], F32R, tag="v", name="v_sb")
            ctxt = [
                ctxt_pool.tile([P, T], F32R, tag=f"c{p}", name=f"ctxt{p}")
                for p in range(NPAIR)
            ]

            def _qk_half(cell, rc, pair, which, lo, hi):
                w_sb, dst = (wq_sb, qt[pair]) if which == 0 else (wk_sb, kt[pair])
                if lo == 0:
                    cell["ps"] = ps_px.tile(
                        [P, QW], F32, tag="px", name=f"qk{rc}{pair}{which}"
                    )
                ps = cell["ps"]
                for fc in range(lo, hi):
                    nc.tensor.matmul(
                        ps,
                        lhsT=w_sb[:, pair, fc, :],
                        rhs=xt_rc[rc][:, fc, :],
                        start=(fc == 0),
                        stop=(fc == FC - 1),
                    )
                if hi == FC:
                    nc.vector.tensor_copy(dst[:, rc * QW : (rc + 1) * QW], ps)

            def emit_qk_one(rc, pair, which):
                cell = {}
                _qk_half(cell, rc, pair, which, 0, 4)
                _qk_half(cell, rc, pair, which, 4, FC)

            def enq_qk(rc, pair, which):
                cell = {}
                projq.append(
                    (None, lambda: _qk_half(cell, rc, pair, which, 0, 4), 852)
                )
                projq.append(
                    (
                        ("qk", rc, pair, which),
                        lambda: _qk_half(cell, rc, pair, which, 4, FC),
                        852,
                    )
                )

            def _v_half(cell, kc, lo, hi):
                rc, ko = divmod(kc, 4)
                if lo == 0:
                    cell["ps"] = ps_px.tile([P, CW], F32, tag="px", name=f"vps{kc}")
                ps = cell["ps"]
                for fc in range(lo, hi):
                    nc.tensor.matmul(
                        ps,
                        lhsT=xt_rc[rc][:, fc, ko * P : (ko + 1) * P],
                        rhs=wv_sb[:, fc, :],
                        start=(fc == 0),
                        stop=(fc == FC - 1),
                    )
                if hi == FC:
                    # all 4 heads' 64 cols in one strided copy (g-step 65)
                    nc.vector.tensor_copy(
                        v_sb[:, kc, :]
                        .rearrange("p (g x) -> p g x", g=4)[:, :, 0:DK],
                        ps.rearrange("p (g x) -> p g x", g=4),
                    )

            def emit_v_kc(kc):
                cell = {}
                _v_half(cell, kc, 0, 4)
                _v_half(cell, kc, 4, FC)

            def enq_v(kc):
                cell = {}
                projq.append((None, lambda: _v_half(cell, kc, 0, 4), 428))
                projq.append((("v", kc), lambda: _v_half(cell, kc, 4, FC), 428))

            def emit_ones_cols():
                # memset (no DMA dependency, unlike sourcing from mask2 —
                # a mask2 wait here would head-block the whole DVE queue)
                for g in range(4):
                    nc.vector.memset(
                        v_sb[:, :, g * 65 + DK : g * 65 + DK + 1], 1.0
                    )

            def emit_drain_pair(qn, pair, ctx_ps, den97, scalar_den=False):
                # drain unnormalized ctx to bf16 SBUF and the denominator
                # rows to 32-aligned partitions of the shared den tile, so
                # the PSUM accumulators free up immediately. For the final
                # pair the den copies ride ScalarE (idle by then) so they
                # overlap the DVE ctx casts on the endgame critical path.
                for he in range(2):
                    nc.vector.tensor_copy(
                        ctxt[pair][
                            he * DK : (he + 1) * DK, qn * QW : (qn + 1) * QW
                        ],
                        ctx_ps[he][0:DK, :],
                    )
                    r = 32 * (2 * pair + he)
                    dstd = den97[r : r + 1, :]
                    srcd = ctx_ps[he][DK : DK + 1, :]
                    if scalar_den:
                        nc.scalar.activation(out=dstd, in_=srcd, func=Copy)
                    else:
                        nc.vector.tensor_copy(dstd, srcd)

            rcp_tiles = {}

            def emit_norm_rcp_pair(qn, pair, den97):
                # 1/den for the pair's two heads (rows 0/32), bf16 output.
                # Junk rows between them stay finite (den buffers are memset
                # to 1.0 once) and the selection matmul zeroes their
                # contribution. Mid-kernel the reciprocal runs on the idle
                # GpSimd so the ScalarE exp stream isn't interrupted; the
                # latency-critical last qn uses ScalarE ln/exp (exp is done).
                rcp33 = rcp_pool.tile(
                    [33, QW], F32R, tag=f"rcp{pair}", name=f"rcp{qn}{pair}"
                )
                den33 = den97[64 * pair : 64 * pair + 33, :]
                lnd = rcp_pool.tile(
                    [33, QW], F32, tag=f"lnd{pair}", name=f"lnd{qn}{pair}"
                )
                nc.scalar.activation(out=lnd, in_=den33, func=Log)
                nc.scalar.activation(out=rcp33, in_=lnd, func=Exp, scale=-1.0)
                rcp_tiles[(qn, pair)] = rcp33

            def emit_norm_mul_pair(qn, pair):
                # bf16 selection matmul broadcasts rcp rows 0/32 across the
                # two heads' 64 partitions each (~0.3us vs ~2.7us for the old
                # per-he K=1 fp32 matmuls), then one in-place multiply for
                # the whole pair tile
                rcp33 = rcp_tiles.pop((qn, pair))
                bc = ps_px.tile([P, QW], F32, tag="px", name=f"bc{qn}{pair}")
                nc.tensor.matmul(bc, lhsT=sel_sb, rhs=rcp33, start=True, stop=True)
                dst = ctxt[pair][:, qn * QW : (qn + 1) * QW]
                nc.vector.tensor_mul(dst, dst, bc)

            def _outproj_half(cell, qn, i2, c2):
                rc = qn * 4 + i2
                if c2 == 0:
                    cell["ob"] = ob_pool.tile([P, D], F32R, tag="ob", name=f"ob{rc}")
                ob = cell["ob"]
                ps = ps_px.tile([P, QW], F32, tag="px", name=f"o{rc}{c2}")
                for pair in range(NPAIR):
                    nc.tensor.matmul(
                        ps,
                        lhsT=ctxt[pair][:, rc * P : (rc + 1) * P],
                        rhs=wo_sb[pair][:, c2 * QW : (c2 + 1) * QW],
                        start=(pair == 0),
                        stop=(pair == NPAIR - 1),
                    )
                dstc = ob[:, c2 * QW : (c2 + 1) * QW]
                if qn == QN - 1:
                    # endgame: alternate casts across ScalarE/DVE and DMA
                    # each 512-col half immediately so the final drain is
                    # short
                    if c2 == 0:
                        nc.scalar.activation(out=dstc, in_=ps, func=Copy)
                    else:
                        nc.vector.tensor_copy(dstc, ps)
                    # alternate the 8 final half-block DMAs across the sync
                    # and scalar trigger queues so the last blocks drain in
                    # parallel instead of serializing on one queue
                    dma_eng = nc.sync if (i2 + c2) % 2 == 0 else nc.scalar
                    dma_eng.dma_start(
                        out=out_d[rc * P : (rc + 1) * P, c2 * QW : (c2 + 1) * QW],
                        in_=dstc,
                    )
                else:
                    nc.vector.tensor_copy(dstc, ps)
                    if c2 == 1:
                        nc.sync.dma_start(
                            out=out_d[rc * P : (rc + 1) * P, :], in_=ob
                        )

            def enq_outproj(qn, i2):
                cell = {}
                slackq.append((lambda: _outproj_half(cell, qn, i2, 0), 426))
                slackq.append((lambda: _outproj_half(cell, qn, i2, 1), 426))

            # ---- flat software-pipelined attention stream ----
            # The ScalarE exp stream is the bottleneck; S matmuls run one
            # group ahead of the ctx matmuls so exp(g+1) never waits on PE
            # work that is queued behind ctx(g). Projections for qn+1, the
            # normalization, and the output projection are interleaved as
            # "filler" slices between attention groups so the PE/DVE queues
            # stay dense (HAM-warm) without starving the exp pipeline.
            from collections import deque

            projq = deque()   # QK/V projection slices: gate later attention
            slackq = deque()  # norm + out-proj slices: no downstream deadline
            emitted = set()

            def pop_one():
                # returns the PE-time estimate (ns) of the popped slice
                if projq:
                    key, fn, cost = projq.popleft()
                    fn()
                    emitted.add(key)
                    return cost
                if slackq:
                    fn, cost = slackq.popleft()
                    fn()
                    return cost
                return 0

            def pops(rem_groups):
                # meter filler by estimated PE time per attention group: the
                # ScalarE exp stream paces a group at ~2.1us of which S+ctx
                # take ~1.3us of PE; overshooting filler in one group starves
                # the exp stream (it waits on the next S behind the filler).
                # Deadline pressure: this qn's queued projections must land
                # before the next qn's attention starts.
                pcost = sum(e[2] for e in projq)
                budget = max(750, pcost // max(rem_groups, 1))
                spent = pops_carry[0]
                n = 0
                while (projq or slackq) and spent < budget and n < 8:
                    spent += pop_one()
                    n += 1
                pops_carry[0] = max(0, spent - budget)

            pops_carry = [0]

            def need(*keys):
                # drain proj fillers until all producer keys are emitted:
                # Tile derives dependencies from trace order, so a consumer
                # must never be traced before its producer
                for k in keys:
                    while k not in emitted:
                        key, fn, cost = projq.popleft()
                        fn()
                        emitted.add(key)

            # prologue: fc0-3 of all four QK projections first (they only
            # need the first xt half), then fc4-7 (second xt half), then V
            # kc0/1 (wv arrives last) — ordered by DMA arrival so the
            # in-order PE queue never head-blocks on a later transfer
            def jitter_dummies(n, label):
                # dependency-free matmuls between DMA-gated segments: absorb
                # input-DMA jitter without idling the PE (an idle PE triggers
                # a HAM downshift to 4/8 duty that outlasts the bubble)
                for wi in range(n):
                    wps = ps_s.tile(
                        [P, 2 * QW], F32, tag="s", name=f"jd_{label}_{wi}"
                    )
                    nc.tensor.matmul(
                        wps[:, 0:QW],
                        lhsT=warm_sb[:, 0:P],
                        rhs=warm_sb,
                        start=True,
                        stop=True,
                    )

            qk_cells = {}
            for pair in range(2):
                for w in range(2):
                    qk_cells[(pair, w)] = {}
                    _qk_half(qk_cells[(pair, w)], 0, pair, w, 0, 4)
                    jitter_dummies(2, f"p{pair}{w}")
            # second xt half lands ~3us after the fc0-3 work drains (the
            # input phase is aggregate-HBM-bound); bridge with dummies so
            # the clock never downshifts
            jitter_dummies(8, "xt0b")
            for pair in range(2):
                for w in range(2):
                    _qk_half(qk_cells[(pair, w)], 0, pair, w, 4, FC)
                    emitted.add(("qk", 0, pair, w))
            for kc in range(2):
                emit_v_kc(kc)
                emitted.add(("v", kc))
            emit_ones_cols()
            # den buffers start finite (1.0) so Reciprocal of junk rows
            # can't produce NaN that the selection matmul would 0*NaN
            for i in range(2):
                nc.vector.memset(
                    rcp_pool.tile([97, QW], F32, tag="den", name=f"den_init{i}"),
                    1.0,
                )
            for kc in (2, 3):
                enq_v(kc)

            flat = []
            for qn in range(QN):
                for pair in range(NPAIR):
                    nkc = 4 * (qn + 1)
                    for kc2 in range(0, nkc, 2):
                        flat.append((qn, pair, kc2, nkc))

            s_tiles = {}

            def emit_S(i):
                qn, pair, kc2, nkc = flat[i]
                need(*[("qk", r, pair, w) for r in range(qn + 1) for w in range(2)])
                sp = {
                    he: ps_s.tile([P, 2 * QW], F32, tag="s", name=f"s{i}{he}")
                    for he in range(2)
                }
                s_tiles[i] = sp
                # he0/he1 interleaved: the K=64 matmuls land on PE row groups
                # 0-63 / 64-127 back-to-back (concurrent row tiling).
                # Diagonal chunks only produce scores for queries >= 128j
                # (the rest is causally masked), so trim the moving width.
                for half in range(2):
                    kc = kc2 + half
                    qoff = max(0, (kc - 4 * qn) * P)
                    for he in range(2):
                        nc.tensor.matmul(
                            sp[he][:, half * QW + qoff : (half + 1) * QW],
                            lhsT=kt[pair][
                                he * DK : (he + 1) * DK, kc * P : (kc + 1) * P
                            ],
                            rhs=qt[pair][
                                he * DK : (he + 1) * DK,
                                qn * QW + qoff : (qn + 1) * QW,
                            ],
                            start=True,
                            stop=True,
                        )

            ctx_tiles = {}
            den_tiles = {}
            mask_alt = [0]
            emit_S(0)
            for i, (qn, pair, kc2, nkc) in enumerate(flat):
                if kc2 == 0:
                    ctx_tiles[(qn, pair)] = {
                        he: ps_ctx.tile(
                            [65, QW], F32, tag=f"x{he}", name=f"ctx{qn}{pair}{he}"
                        )
                        for he in range(2)
                    }
                    if pair == 0:
                        den_tiles[qn] = rcp_pool.tile(
                            [97, QW], F32, tag="den", name=f"den{qn}"
                        )
                        if qn + 1 < QN:
                            for p2 in range(NPAIR):
                                for w in range(2):
                                    enq_qk(qn + 1, p2, w)
                            for kc in range(4 * (qn + 1), 4 * (qn + 1) + 4):
                                enq_v(kc)
                ctx_ps = ctx_tiles[(qn, pair)]
                sp = s_tiles.pop(i)
                dg = 0 if kc2 == 4 * qn else (1 if kc2 == 4 * qn + 2 else -1)
                goff = max(0, (kc2 - 4 * qn) * P)  # group exp start col
                es = {}
                for he in range(2):
                    e = exp_pool.tile([P, 2 * QW], F32R, tag="exp", name=f"e{i}{he}")
                    nc.scalar.activation(
                        out=e[:, goff:], in_=sp[he][:, goff:], func=Exp, scale=0.125
                    )
                    if dg >= 0:
                        eng = nc.vector
                        if GPSIMD_MASKS:
                            mask_alt[0] ^= 1
                            if mask_alt[0]:
                                eng = nc.gpsimd
                        eng.tensor_mul(
                            e[:, goff:],
                            e[:, goff:],
                            mask2_sb[:, dg * 2 * QW + goff : (dg + 1) * 2 * QW],
                        )
                    es[he] = e
                if i + 1 < len(flat):
                    emit_S(i + 1)
                need(("v", kc2), ("v", kc2 + 1))
                for half in range(2):
                    kc = kc2 + half
                    qoff = max(0, (kc - 4 * qn) * P)
                    for he in range(2):
                        g = 2 * pair + he
                        nc.tensor.matmul(
                            ctx_ps[he][:, qoff:],
                            lhsT=v_sb[:, kc, g * 65 : g * 65 + 65],
                            rhs=es[he][:, half * QW + qoff : (half + 1) * QW],
                            start=(kc == 0),
                            stop=(kc == nkc - 1),
                        )
                if kc2 == nkc - 2:
                    last = qn == QN - 1 and pair == NPAIR - 1
                    emit_drain_pair(
                        qn, pair, ctx_ps, den_tiles[qn], scalar_den=last
                    )
                    del ctx_tiles[(qn, pair)]
                    d97 = den_tiles[qn]
                    slackq.append(
                        (
                            lambda qn=qn, p=pair, d=d97: emit_norm_rcp_pair(
                                qn, p, d
                            ),
                            0,
                        )
                    )
                    slackq.append(
                        (lambda qn=qn, p=pair: emit_norm_mul_pair(qn, p), 213)
                    )
                    if pair == NPAIR - 1:
                        for i2 in range(4):
                            enq_outproj(qn, i2)
                pops(nkc - (pair * (nkc // 2) + kc2 // 2))
            # endgame HAM keep-warm: the final norm chain leaves the PE
            # briefly idle, which would downshift the clock to 4/8 duty for
            # the whole output projection; dependency-free dummy matmuls
            # bridge the gap (ps_s banks are free after the last exp)
            for wi in range(15):
                wps = ps_s.tile([P, 2 * QW], F32, tag="s", name=f"ewarm{wi}")
                nc.tensor.matmul(
                    wps[:, 0:QW],
                    lhsT=warm_sb[:, 0:P],
                    rhs=warm_sb,
                    start=True,
                    stop=True,
                )
            while projq or slackq:
                pop_one()

    _split_multi_waits(nc)
    _PROGRAM = nc
    return nc


def _make_mask2():
    # mask2[:, 512j:512j+512] = stair(j): [k, q] = 1.0 iff q >= 128j + k
    k = np.arange(P)[:, None]
    q = np.arange(QW)[None, :]
    blocks = [(q >= 128 * j + k).astype(np.float32) for j in range(4)]
    return np.concatenate(blocks, axis=1)


def make_in_maps(x, Wq, Wk, Wv, Wo):
    import ml_dtypes

    nd = ml_dtypes.bfloat16 if F32R == BF16 else np.float32
    x = np.asarray(x, dtype=np.float32)
    mask2 = _make_mask2().astype(nd)
    # selection matrix for the softmax-normalization broadcast matmul:
    # bc[m] = rcp[0] for m<64 (he0), rcp[32] for m>=64 (he1)
    sel = np.zeros((33, P), np.float32)
    sel[0, 0:DK] = 1.0
    sel[32, DK:P] = 1.0
    sel = sel.astype(nd)
    Wq, Wk, Wv, Wo = (np.asarray(w, dtype=np.float32) for w in (Wq, Wk, Wv, Wo))
    # blocked x^T: [p, rc, f, c] so device rc-slices are contiguous
    xts = [
        np.ascontiguousarray(
            x[b].reshape(QN, QW, FC, P).transpose(3, 0, 2, 1).reshape(P, -1)
        ).astype(nd)
        for b in range(B)
    ]
    def qk_block(W, cols):
        # [p][pair][f][c]: per-pair slices are contiguous 2KB/partition
        s = W[:, cols].reshape(FC, P, NPAIR, P)
        return np.ascontiguousarray(
            s.transpose(1, 2, 0, 3).reshape(P, -1)
        ).astype(nd)

    def v_block(W, cols):
        # [p][f][c]: one contiguous 4KB run per partition
        s = W[:, cols].reshape(FC, P, CW)
        return np.ascontiguousarray(s.transpose(1, 0, 2).reshape(P, -1)).astype(nd)

    in_maps = []
    for c in range(NCORES):
        b, q4 = divmod(c, NCORES // B)
        cols = slice(q4 * CW, (q4 + 1) * CW)
        in_maps.append(
            {
                "xt": xts[b],
                "wq": qk_block(Wq, cols),
                "wk": qk_block(Wk, cols),
                "wv": v_block(Wv, cols),
                "wo": np.ascontiguousarray(Wo[cols, :]).astype(nd),
                "mask2": mask2,
                "sel": sel,
            }
        )
    return in_maps


def reduce_outputs(results):
    """Sum the per-core bf16 partials (4 cores per batch) in f64."""
    out = np.zeros((B, T, D), dtype=np.float64)
    for c in range(NCORES):
        b = c // (NCORES // B)
        out[b] += np.asarray(results[c]["out"], dtype=np.float64)
    return out.astype(np.float32)


def kernel(x, Wq, Wk, Wv, Wo):
    from concourse.bass_utils import run_bass_kernel_spmd

    nc = build_program()
    in_maps = make_in_maps(x, Wq, Wk, Wv, Wo)
    res = run_bass_kernel_spmd(nc, in_maps, core_ids=list(range(NCORES)))
    return reduce_outputs(res.results)


if __name__ == "__main__":
    rng = np.random.default_rng(0)
    s = 1.0 / np.sqrt(D)
    ins = {
        "x": rng.standard_normal((B, T, D)).astype(np.float32),
        "Wq": (rng.standard_normal((D, D)) * s).astype(np.float32),
        "Wk": (rng.standard_normal((D, D)) * s).astype(np.float32),
        "Wv": (rng.standard_normal((D, D)) * s).astype(np.float32),
        "Wo": (rng.standard_normal((D, D)) * (1.0 / np.sqrt(D))).astype(np.float32),
    }
    out = kernel(**ins)
    print("out", out.shape, out.dtype, float(np.abs(out).max()))



# revision 57
# speedup vs baseline: 1.0150x; 1.0082x over previous
"""Trainium2 Bass kernel for 16-head causal MHA (B=2, T=2048, D=1024, fp32 I/O).

Sharding: core c owns batch c//4 and head-quad c%4 (heads 4q..4q+3, as two
head-pairs). It computes Q/K/V projections for its 256 q/k/v dims, causal
attention for its 4 heads, and a partial output [2048, 1024] (bf16); the host
sums the 4 partials per batch in f64. One batch per core halves the partial-
output drain (PSUM->SBUF casts + DMA) and the x^T input DMA vs 2-batch cores.

Per-core device program, matmul inputs bf16 (2 cols/PE-cycle + fast weight
load), fp32 PSUM accumulation. The PE matmul stream is the bottleneck
(~137us busy of ~158us wall); the schedule keeps it dense from the first
projection to the last output block:
  - Q^T, K^T = W.T @ x^T (weights stationary, N=512 moving blocks)
  - V natural = x @ Wv with a ones column per head (denominator for free);
    one V tile holds all 4 heads so each chunk drains in one strided copy
  - attention in S^T layout per head-pair: the two heads' K=64 contractions
    sit at PE base partitions 0/64 (row-tiled, concurrent); exp on ScalarE
    with the 1/sqrt(dk) scale folded in; causality = skipping fully-masked
    blocks + one [128,1024] staircase multiply per diagonal 2-chunk group
    (mask2 packs the 4 staircase patterns contiguously)
  - normalization: denominator rows drain to 32-aligned partitions of a
    [97,512] tile per qn (memset to 1.0 once so junk rows stay finite),
    1/den = exp(-ln(den)) per pair in two ScalarE [33,512] calls to bf16,
    then ONE bf16 K=33 selection matmul per pair broadcasts the two rcp
    rows across the heads' partitions (replaces the old per-he K=1 fp32
    matmuls, ~20us of PE) and one in-place multiply normalizes the pair
  - partial out per 128-query chunk: two accumulating K=128 matmuls (one
    per head-pair), cast to bf16, DMA'd per 1024-col row block; the last
    qn casts alternate ScalarE/DVE and DMA per 512-col half
  - diagonal-block S/ctx matmuls and exp trim their moving width to the
    causally-live queries (the masked region is never computed)
  - x^T and the weights are host-pre-blocked so every DMA reads contiguous
    multi-KB runs, spread over the sync/scalar/gpsimd trigger queues in
    consumption order (the input phase is aggregate-HBM-bound at
    ~358GB/s); x^T arrives per 512-token block so attention starts early
  - attention-phase filler (later projections, out-proj, norm) is metered
    by estimated PE-time per group so the ScalarE exp stream never starves
    behind a filler burst, with deadline pressure from the next qn's needs;
    the base budget is deliberately below the average filler demand
    (750ns vs ~1.1us) so filler defers toward its deadline and the next
    S tile enters the PE queue right as exp frees its PSUM slot
    (950->750 measured ~1.5us; 600 showed no further gain)
  - the PE is never allowed to idle >1us: dependency-free dummy matmuls
    bridge DMA jitter in the prologue and the norm chain in the endgame,
    because an idle PE triggers a HAM clock downshift to 4/8 duty that
    roughly doubles matmul time until well after the bubble ends. 15
    endgame dummies is the tuned count: 10 leaves a downshift-triggering
    gap, 23 outlasts the norm window and delays the first out-proj MMs
    (the dummies serialize pairwise on the two s-tag PSUM slots).

Infrastructure: the external walrus allows only ONE sync wait per
instruction; a post-pass hoists extra waits onto single-wait no-ops and the
TileContext closing drain is split into a chain of single-wait drains. The
closing drain also skips the device-side semaphore/dma reset + second
barrier (~7us after the last DMA; the NEFF executes once per kernel()
call). Known-bad variants (do not retry): GpSimd cannot touch PSUM (den
copies must stay on DVE/ScalarE); custom-DVE ops (reciprocal_approx_fast)
fail this walrus ("ISA wrong length"); GpSimd tensor_mul for the staircase
masks is too slow and stalls the exp->ctx chain; reusing the freed ps_ctx
banks for the endgame out-proj accumulators corrupts the output. Note
~±2-30us run-to-run device clock variance on these cores - compare
schedules by the min of several runs.
"""

import numpy as np

import bass_rust
from bass_rust import ScopedClock
import concourse.bass as bass
import concourse.mybir as mybir
import concourse.tile as tile

F32 = mybir.dt.float32
BF16 = mybir.dt.bfloat16
F32R = BF16
B, T, D = 2, 2048, 1024
NCORES = 8
P = 128          # partitions / feature chunk
FC = D // P      # 8 feature chunks
QW = 512         # query block width (PSUM bank)
QN = T // QW     # 4 query blocks
KC = T // P      # 16 key chunks
DK = 64
CW = 256         # q/k/v dims per core (4 heads x 64)
NPAIR = 2        # head-pairs per core

# Set True to offload half the diagonal-mask multiplies to GpSimd.
GPSIMD_MASKS = False

# ---------------------------------------------------------------------------
# TileContext drain fix: the external walrus in this container allows only ONE
# sync wait per instruction, but Tile's closing drain packs one wait per active
# proc. Split it into a chain of single-wait drains (same semantics).
_PATCHED = False


def _patched_drain_and_barrier(self, tick_clock, wait_clock):
    nc = self.nc
    drain_inst = nc.sync.drain()
    wait_clock.add_sem_waits(
        drain_inst.ins, ScopedClock({None: tick_clock.global_clock})
    )
    si = drain_inst.ins.sync_info
    waits = list(si.on_wait) if si is not None else []
    if len(waits) > 1:
        si.on_wait = [waits[0]]
        drain_inst.ins.sync_info = si
        for w in waits[1:]:
            d2 = nc.sync.drain()
            si2 = d2.ins.sync_info
            if si2 is None:
                si2 = bass_rust.SyncInfo(on_wait=[w], on_update=[])
            else:
                si2.on_wait = [w]
            d2.ins.sync_info = si2
    nc.all_engine_barrier()
    assert self.sems is not None
    popped = nc._tile_sem_poison_stack.pop()
    assert popped is self._sem_poison
    # End of program: skip the device-side semaphore/dma-queue reset and the
    # second barrier (the trace shows that cascade costs ~7us after the last
    # DMA completes). The NEFF executes once per kernel() call, so nothing
    # re-reads the dirty semaphores.


def _apply_tile_patch():
    global _PATCHED
    if not _PATCHED:
        tile.TileContext._drain_and_barrier = _patched_drain_and_barrier
        _PATCHED = True


def _split_multi_waits(nc):
    """Post-pass: the external walrus accepts only 1 sync wait per
    instruction (2 for EventSemaphore). Tile emits more. Hoist extra waits
    onto same-engine no-ops inserted just before. For compute engines this
    is identical semantics (the engine blocks either way). For DMA triggers
    it turns queue-side waits into SP-side blocking, which is safe in this
    forward-dataflow single-block program (every wait's producer precedes
    the trigger in the scheduled stream); CoreSim re-validates no-deadlock."""
    for f in nc.m.functions:
        for bb in f.blocks:
            new = []
            for ins in bb.instructions:
                si = ins.sync_info
                if si is not None:
                    cap = 2 if isinstance(ins, mybir.InstEventSemaphore) else 1
                    waits = list(si.on_wait)
                    if len(waits) > cap:
                        for w in waits[:-cap]:
                            nop = mybir.InstNoOp(
                                name=nc.get_next_instruction_name(),
                                engine=ins.engine,
                                sync_info=bass_rust.SyncInfo(
                                    on_wait=[w], on_update=[]
                                ),
                                bass_nofuse=True,
                            )
                            nc.register_instruction(nop, overwrite=True)
                            new.append(nop)
                        si.on_wait = waits[-cap:]
                        ins.sync_info = si
                new.append(ins)
            bb.instructions = new


# ---------------------------------------------------------------------------
_PROGRAM = None


def build_program():
    global _PROGRAM
    if _PROGRAM is not None:
        return _PROGRAM
    _apply_tile_patch()
    Exp = mybir.ActivationFunctionType.Exp
    Log = mybir.ActivationFunctionType.Ln
    Copy = mybir.ActivationFunctionType.Copy

    nc = bass.Bass()
    xt_d = nc.declare_dram_parameter("xt", [P, QN * FC * QW], F32R, isOutput=False)
    wq_d = nc.declare_dram_parameter("wq", [P, NPAIR * FC * P], F32R, isOutput=False)
    wk_d = nc.declare_dram_parameter("wk", [P, NPAIR * FC * P], F32R, isOutput=False)
    wv_d = nc.declare_dram_parameter("wv", [P, FC * CW], F32R, isOutput=False)
    wo_d = nc.declare_dram_parameter("wo", [CW, D], F32R, isOutput=False)
    mask2_d = nc.declare_dram_parameter("mask2", [P, 4 * QW], F32R, isOutput=False)
    sel_d = nc.declare_dram_parameter("sel", [33, P], F32R, isOutput=False)
    out_d = nc.declare_dram_parameter("out", [T, D], F32R, isOutput=True)

    with tile.TileContext(nc) as tc:
        from contextlib import ExitStack

        ctx = ExitStack()
        with ctx:
            consts = ctx.enter_context(tc.tile_pool(name="consts", bufs=1))
            xt_pool = ctx.enter_context(tc.tile_pool(name="xt", bufs=1))
            qk_pool = ctx.enter_context(tc.tile_pool(name="qk", bufs=1))
            v_pool = ctx.enter_context(tc.tile_pool(name="v", bufs=1))
            exp_pool = ctx.enter_context(tc.tile_pool(name="exp", bufs=8))
            ctxt_pool = ctx.enter_context(tc.tile_pool(name="ctxt", bufs=1))
            rcp_pool = ctx.enter_context(tc.tile_pool(name="rcp", bufs=2))
            ob_pool = ctx.enter_context(tc.tile_pool(name="ob", bufs=3))

            ps_s = ctx.enter_context(tc.tile_pool(name="ps_s", bufs=2, space="PSUM"))
            ps_ctx = ctx.enter_context(
                tc.tile_pool(name="ps_ctx", bufs=1, space="PSUM")
            )
            ps_px = ctx.enter_context(tc.tile_pool(name="ps_px", bufs=2, space="PSUM"))

            # ---- constants ----
            wq_sb = consts.tile([P, NPAIR, FC, P], F32R, tag="wq")
            wk_sb = consts.tile([P, NPAIR, FC, P], F32R, tag="wk")
            wv_sb = consts.tile([P, FC, CW], F32R, tag="wv")
            wo_sb = [
                consts.tile([P, D], F32R, tag=f"wo{p}", name=f"wo_sb{p}")
                for p in range(NPAIR)
            ]
            mask2_sb = consts.tile([P, 4 * QW], F32R, tag="mask2")
            sel_sb = consts.tile([33, P], F32R, tag="sel")
            # DMA triggers spread across engine queues so the ~0.6us
            # per-trigger cost parallelizes and x^T lands ASAP
            # HAM warm-up: ~5us of dummy matmuls on a memset tile flip the
            # PE clock gate to 8/8 during the input-DMA window, so the real
            # projection prologue runs at 2.4 GHz instead of 1.2
            warm_sb = consts.tile([P, QW], F32R, tag="warm")
            nc.vector.memset(warm_sb, 0.0)
            for wi in range(10):
                wps = ps_px.tile([P, QW], F32, tag="px", name=f"warm{wi}")
                nc.tensor.matmul(
                    wps, lhsT=warm_sb[:, 0:P], rhs=warm_sb, start=True, stop=True
                )

            # x^T arrives by query/key block: qn0's attention needs only
            # tokens 0-511. All inputs are host-pre-blocked so every DMA
            # reads contiguous multi-KB runs per partition at full rate.
            # Queue plan orders each trigger queue by when data is needed;
            # xt block 1 rides the otherwise-idle vector queue so it lands
            # before qn1's attention (~22us) instead of behind wv.
            # one tile per 512-token block: precise DMA->matmul dependencies
            # so attention on early blocks never waits on later blocks' DMAs
            xt_rc = [
                xt_pool.tile([P, FC, QW], F32R, tag=f"xt{rc}", name=f"xt_rc{rc}")
                for rc in range(QN)
            ]

            def xt_blk(rc, fclo, fchi):
                return xt_d[
                    :, rc * FC * QW + fclo * QW : rc * FC * QW + fchi * QW
                ].rearrange("p (f c) -> p f c", c=QW)

            def w_blk(w_d, pair):
                return w_d[
                    :, pair * FC * P : (pair + 1) * FC * P
                ].rearrange("p (f c) -> p f c", c=P)

            # queue plan (all three trigger queues contend for ~358GB/s of
            # HBM, per-queue throughput is arbitration-dependent): spread the
            # early-needed tensors across the queues in consumption order so
            # no single queue's crawl stalls the projection prologue
            nc.sync.dma_start(out=wq_sb[:, 0], in_=w_blk(wq_d, 0))
            nc.scalar.dma_start(out=xt_rc[0][:, 0:4, :], in_=xt_blk(0, 0, 4))
            nc.gpsimd.dma_start(out=wk_sb[:, 0], in_=w_blk(wk_d, 0))
            nc.scalar.dma_start(out=wq_sb[:, 1], in_=w_blk(wq_d, 1))
            nc.gpsimd.dma_start(out=xt_rc[0][:, 4:FC, :], in_=xt_blk(0, 4, FC))
            nc.scalar.dma_start(out=wk_sb[:, 1], in_=w_blk(wk_d, 1))
            nc.sync.dma_start(
                out=wv_sb, in_=wv_d.rearrange("p (f c) -> p f c", c=CW)
            )
            nc.scalar.dma_start(out=xt_rc[1][:, 0:4, :], in_=xt_blk(1, 0, 4))
            nc.scalar.dma_start(out=xt_rc[1][:, 4:FC, :], in_=xt_blk(1, 4, FC))
            nc.sync.dma_start(out=mask2_sb, in_=mask2_d[:, :])
            nc.scalar.dma_start(out=sel_sb, in_=sel_d[:, :])
            nc.gpsimd.dma_start(out=xt_rc[2][:, 0:4, :], in_=xt_blk(2, 0, 4))
            nc.gpsimd.dma_start(out=xt_rc[2][:, 4:FC, :], in_=xt_blk(2, 4, FC))
            nc.gpsimd.dma_start(out=xt_rc[3], in_=xt_blk(3, 0, FC))
            for p in range(NPAIR):
                nc.sync.dma_start(out=wo_sb[p], in_=wo_d[p * P : (p + 1) * P, :])

            qt = [
                qk_pool.tile([P, T], F32R, tag=f"qt{p}", name=f"qt{p}")
                for p in range(NPAIR)
            ]
            kt = [
                qk_pool.tile([P, T], F32R, tag=f"kt{p}", name=f"kt{p}")
                for p in range(NPAIR)
            ]
            # one V tile for all 4 heads: group g=2*pair+he at cols [g*65,
            # g*65+65) per kc (64 v-dims + the ones/denominator column)
            v_sb = v_pool.tile([P, KC, 4 * 65], F32R, tag="v", name="v_sb")
            ctxt = [
                ctxt_pool.tile([P, T], F32R, tag=f"c{p}", name=f"ctxt{p}")
                for p in range(NPAIR)
            ]

            def _qk_half(cell, rc, pair, which, lo, hi):
                w_sb, dst = (wq_sb, qt[pair]) if which == 0 else (wk_sb, kt[pair])
                if lo == 0:
                    cell["ps"] = ps_px.tile(
                        [P, QW], F32, tag="px", name=f"qk{rc}{pair}{which}"
                    )
                ps = cell["ps"]
                for fc in range(lo, hi):
                    nc.tensor.matmul(
                        ps,
                        lhsT=w_sb[:, pair, fc, :],
                        rhs=xt_rc[rc][:, fc, :],
                        start=(fc == 0),
                        stop=(fc == FC - 1),
                    )
                if hi == FC:
                    nc.vector.tensor_copy(dst[:, rc * QW : (rc + 1) * QW], ps)

            def emit_qk_one(rc, pair, which):
                cell = {}
                _qk_half(cell, rc, pair, which, 0, 4)
                _qk_half(cell, rc, pair, which, 4, FC)

            def enq_qk(rc, pair, which):
                cell = {}
                projq.append(
                    (None, lambda: _qk_half(cell, rc, pair, which, 0, 4), 852)
                )
                projq.append(
                    (
                        ("qk", rc, pair, which),
                        lambda: _qk_half(cell, rc, pair, which, 4, FC),
                        852,
                    )
                )

            def _v_half(cell, kc, lo, hi):
                rc, ko = divmod(kc, 4)
                if lo == 0:
                    cell["ps"] = ps_px.tile([P, CW], F32, tag="px", name=f"vps{kc}")
                ps = cell["ps"]
                for fc in range(lo, hi):
                    nc.tensor.matmul(
                        ps,
                        lhsT=xt_rc[rc][:, fc, ko * P : (ko + 1) * P],
                        rhs=wv_sb[:, fc, :],
                        start=(fc == 0),
                        stop=(fc == FC - 1),
                    )
                if hi == FC:
                    # all 4 heads' 64 cols in one strided copy (g-step 65)
                    nc.vector.tensor_copy(
                        v_sb[:, kc, :]
                        .rearrange("p (g x) -> p g x", g=4)[:, :, 0:DK],
                        ps.rearrange("p (g x) -> p g x", g=4),
                    )

            def emit_v_kc(kc):
                cell = {}
                _v_half(cell, kc, 0, 4)
                _v_half(cell, kc, 4, FC)

            def enq_v(kc):
                cell = {}
                projq.append((None, lambda: _v_half(cell, kc, 0, 4), 428))
                projq.append((("v", kc), lambda: _v_half(cell, kc, 4, FC), 428))

            def emit_ones_cols():
                # memset (no DMA dependency, unlike sourcing from mask2 —
                # a mask2 wait here would head-block the whole DVE queue)
                for g in range(4):
                    nc.vector.memset(
                        v_sb[:, :, g * 65 + DK : g * 65 + DK + 1], 1.0
                    )

            def emit_drain_pair(qn, pair, ctx_ps, den97, scalar_den=False):
                # drain unnormalized ctx to bf16 SBUF and the denominator
                # rows to 32-aligned partitions of the shared den tile, so
                # the PSUM accumulators free up immediately. For the final
                # pair the den copies ride ScalarE (idle by then) so they
                # overlap the DVE ctx casts on the endgame critical path.
                for he in range(2):
                    nc.vector.tensor_copy(
                        ctxt[pair][
                            he * DK : (he + 1) * DK, qn * QW : (qn + 1) * QW
                        ],
                        ctx_ps[he][0:DK, :],
                    )
                    r = 32 * (2 * pair + he)
                    dstd = den97[r : r + 1, :]
                    srcd = ctx_ps[he][DK : DK + 1, :]
                    if scalar_den:
                        nc.scalar.activation(out=dstd, in_=srcd, func=Copy)
                    else:
                        nc.vector.tensor_copy(dstd, srcd)

            rcp_tiles = {}

            def emit_norm_rcp_pair(qn, pair, den97):
                # 1/den for the pair's two heads (rows 0/32), bf16 output.
                # Junk rows between them stay finite (den buffers are memset
                # to 1.0 once) and the selection matmul zeroes their
                # contribution. Mid-kernel the reciprocal runs on the idle
                # GpSimd so the ScalarE exp stream isn't interrupted; the
                # latency-critical last qn uses ScalarE ln/exp (exp is done).
                rcp33 = rcp_pool.tile(
                    [33, QW], F32R, tag=f"rcp{pair}", name=f"rcp{qn}{pair}"
                )
                den33 = den97[64 * pair : 64 * pair + 33, :]
                lnd = rcp_pool.tile(
                    [33, QW], F32, tag=f"lnd{pair}", name=f"lnd{qn}{pair}"
                )
                nc.scalar.activation(out=lnd, in_=den33, func=Log)
                nc.scalar.activation(out=rcp33, in_=lnd, func=Exp, scale=-1.0)
                rcp_tiles[(qn, pair)] = rcp33

            def emit_norm_mul_pair(qn, pair):
                # bf16 selection matmul broadcasts rcp rows 0/32 across the
                # two heads' 64 partitions each (~0.3us vs ~2.7us for the old
                # per-he K=1 fp32 matmuls), then one in-place multiply for
                # the whole pair tile
                rcp33 = rcp_tiles.pop((qn, pair))
                bc = ps_px.tile([P, QW], F32, tag="px", name=f"bc{qn}{pair}")
                nc.tensor.matmul(bc, lhsT=sel_sb, rhs=rcp33, start=True, stop=True)
                dst = ctxt[pair][:, qn * QW : (qn + 1) * QW]
                nc.vector.tensor_mul(dst, dst, bc)

            def _outproj_half(cell, qn, i2, c2):
                rc = qn * 4 + i2
                if c2 == 0:
                    cell["ob"] = ob_pool.tile([P, D], F32R, tag="ob", name=f"ob{rc}")
                ob = cell["ob"]
                ps = ps_px.tile([P, QW], F32, tag="px", name=f"o{rc}{c2}")
                for pair in range(NPAIR):
                    nc.tensor.matmul(
                        ps,
                        lhsT=ctxt[pair][:, rc * P : (rc + 1) * P],
                        rhs=wo_sb[pair][:, c2 * QW : (c2 + 1) * QW],
                        start=(pair == 0),
                        stop=(pair == NPAIR - 1),
                    )
                dstc = ob[:, c2 * QW : (c2 + 1) * QW]
                if qn == QN - 1:
                    # endgame: alternate casts across ScalarE/DVE and DMA
                    # each 512-col half immediately so the final drain is
                    # short
                    if c2 == 0:
                        nc.scalar.activation(out=dstc, in_=ps, func=Copy)
                    else:
                        nc.vector.tensor_copy(dstc, ps)
                    # alternate the 8 final half-block DMAs across the sync
                    # and scalar trigger queues so the last blocks drain in
                    # parallel instead of serializing on one queue
                    dma_eng = nc.sync if (i2 + c2) % 2 == 0 else nc.scalar
                    dma_eng.dma_start(
                        out=out_d[rc * P : (rc + 1) * P, c2 * QW : (c2 + 1) * QW],
                        in_=dstc,
                    )
                else:
                    nc.vector.tensor_copy(dstc, ps)
                    if c2 == 1:
                        nc.sync.dma_start(
                            out=out_d[rc * P : (rc + 1) * P, :], in_=ob
                        )

            def enq_outproj(qn, i2):
                cell = {}
                slackq.append((lambda: _outproj_half(cell, qn, i2, 0), 426))
                slackq.append((lambda: _outproj_half(cell, qn, i2, 1), 426))

            # ---- flat software-pipelined attention stream ----
            # The ScalarE exp stream is the bottleneck; S matmuls run one
            # group ahead of the ctx matmuls so exp(g+1) never waits on PE
            # work that is queued behind ctx(g). Projections for qn+1, the
            # normalization, and the output projection are interleaved as
            # "filler" slices between attention groups so the PE/DVE queues
            # stay dense (HAM-warm) without starving the exp pipeline.
            from collections import deque

            projq = deque()   # QK/V projection slices: gate later attention
            slackq = deque()  # norm + out-proj slices: no downstream deadline
            emitted = set()

            def pop_one():
                # returns the PE-time estimate (ns) of the popped slice
                if projq:
                    key, fn, cost = projq.popleft()
                    fn()
                    emitted.add(key)
                    return cost
                if slackq:
                    fn, cost = slackq.popleft()
                    fn()
                    return cost
                return 0

            def pops(rem_groups):
                # meter filler by estimated PE time per attention group: the
                # ScalarE exp stream paces a group at ~2.1us of which S+ctx
                # take ~1.3us of PE; overshooting filler in one group starves
                # the exp stream (it waits on the next S behind the filler).
                # Deadline pressure: this qn's queued projections must land
                # before the next qn's attention starts.
                pcost = sum(e[2] for e in projq)
                budget = max(750, pcost // max(rem_groups, 1))
                spent = pops_carry[0]
                n = 0
                while (projq or slackq) and spent < budget and n < 8:
                    spent += pop_one()
                    n += 1
                pops_carry[0] = max(0, spent - budget)

            pops_carry = [0]

            def need(*keys):
                # drain proj fillers until all producer keys are emitted:
                # Tile derives dependencies from trace order, so a consumer
                # must never be traced before its producer
                for k in keys:
                    while k not in emitted:
                        key, fn, cost = projq.popleft()
                        fn()
                        emitted.add(key)

            # prologue: fc0-3 of all four QK projections first (they only
            # need the first xt half), then fc4-7 (second xt half), then V
            # kc0/1 (wv arrives last) — ordered by DMA arrival so the
            # in-order PE queue never head-blocks on a later transfer
            def jitter_dummies(n, label):
                # dependency-free matmuls between DMA-gated segments: absorb
                # input-DMA jitter without idling the PE (an idle PE triggers
                # a HAM downshift to 4/8 duty that outlasts the bubble)
                for wi in range(n):
                    wps = ps_s.tile(
                        [P, 2 * QW], F32, tag="s", name=f"jd_{label}_{wi}"
                    )
                    nc.tensor.matmul(
                        wps[:, 0:QW],
                        lhsT=warm_sb[:, 0:P],
                        rhs=warm_sb,
                        start=True,
                        stop=True,
                    )

            qk_cells = {}
            for pair in range(2):
                for w in range(2):
                    qk_cells[(pair, w)] = {}
                    _qk_half(qk_cells[(pair, w)], 0, pair, w, 0, 4)
                    jitter_dummies(2, f"p{pair}{w}")
            # second xt half lands ~3us after the fc0-3 work drains (the
            # input phase is aggregate-HBM-bound); bridge with dummies so
            # the clock never downshifts
            jitter_dummies(8, "xt0b")
            for pair in range(2):
                for w in range(2):
                    _qk_half(qk_cells[(pair, w)], 0, pair, w, 4, FC)
                    emitted.add(("qk", 0, pair, w))
            for kc in range(2):
                emit_v_kc(kc)
                emitted.add(("v", kc))
            emit_ones_cols()
            # den buffers start finite (1.0) so Reciprocal of junk rows
            # can't produce NaN that the selection matmul would 0*NaN
            for i in range(2):
                nc.vector.memset(
                    rcp_pool.tile([97, QW], F32, tag="den", name=f"den_init{i}"),
                    1.0,
                )
            for kc in (2, 3):
                enq_v(kc)

            flat = []
            for qn in range(QN):
                for pair in range(NPAIR):
                    nkc = 4 * (qn + 1)
                    for kc2 in range(0, nkc, 2):
                        flat.append((qn, pair, kc2, nkc))

            s_tiles = {}

            def emit_S(i):
                qn, pair, kc2, nkc = flat[i]
                need(*[("qk", r, pair, w) for r in range(qn + 1) for w in range(2)])
                sp = {
                    he: ps_s.tile([P, 2 * QW], F32, tag="s", name=f"s{i}{he}")
                    for he in range(2)
                }
                s_tiles[i] = sp
                # he0/he1 interleaved: the K=64 matmuls land on PE row groups
                # 0-63 / 64-127 back-to-back (concurrent row tiling).
                # Diagonal chunks only produce scores for queries >= 128j
                # (the rest is causally masked), so trim the moving width.
                for half in range(2):
                    kc = kc2 + half
                    qoff = max(0, (kc - 4 * qn) * P)
                    for he in range(2):
                        nc.tensor.matmul(
                            sp[he][:, half * QW + qoff : (half + 1) * QW],
                            lhsT=kt[pair][
                                he * DK : (he + 1) * DK, kc * P : (kc + 1) * P
                            ],
                            rhs=qt[pair][
                                he * DK : (he + 1) * DK,
                                qn * QW + qoff : (qn + 1) * QW,
                            ],
                            start=True,
                            stop=True,
                        )

            ctx_tiles = {}
            den_tiles = {}
            mask_alt = [0]
            emit_S(0)
            for i, (qn, pair, kc2, nkc) in enumerate(flat):
                if kc2 == 0:
                    ctx_tiles[(qn, pair)] = {
                        he: ps_ctx.tile(
                            [65, QW], F32, tag=f"x{he}", name=f"ctx{qn}{pair}{he}"
                        )
                        for he in range(2)
                    }
                    if pair == 0:
                        den_tiles[qn] = rcp_pool.tile(
                            [97, QW], F32, tag="den", name=f"den{qn}"
                        )
                        if qn + 1 < QN:
                            for p2 in range(NPAIR):
                                for w in range(2):
                                    enq_qk(qn + 1, p2, w)
                            for kc in range(4 * (qn + 1), 4 * (qn + 1) + 4):
                                enq_v(kc)
                ctx_ps = ctx_tiles[(qn, pair)]
                sp = s_tiles.pop(i)
                dg = 0 if kc2 == 4 * qn else (1 if kc2 == 4 * qn + 2 else -1)
                goff = max(0, (kc2 - 4 * qn) * P)  # group exp start col
                es = {}
                for he in range(2):
                    e = exp_pool.tile([P, 2 * QW], F32R, tag="exp", name=f"e{i}{he}")
                    nc.scalar.activation(
                        out=e[:, goff:], in_=sp[he][:, goff:], func=Exp, scale=0.125
                    )
                    if dg >= 0:
                        eng = nc.vector
                        if GPSIMD_MASKS:
                            mask_alt[0] ^= 1
                            if mask_alt[0]:
                                eng = nc.gpsimd
                        eng.tensor_mul(
                            e[:, goff:],
                            e[:, goff:],
                            mask2_sb[:, dg * 2 * QW + goff : (dg + 1) * 2 * QW],
                        )
                    es[he] = e
                if i + 1 < len(flat):
                    emit_S(i + 1)
                need(("v", kc2), ("v", kc2 + 1))
                for half in range(2):
                    kc = kc2 + half
                    qoff = max(0, (kc - 4 * qn) * P)
                    for he in range(2):
                        g = 2 * pair + he
                        nc.tensor.matmul(
                            ctx_ps[he][:, qoff:],
                            lhsT=v_sb[:, kc, g * 65 : g * 65 + 65],
                            rhs=es[he][:, half * QW + qoff : (half + 1) * QW],
                            start=(kc == 0),
                            stop=(kc == nkc - 1),
                        )
                if kc2 == nkc - 2:
                    last = qn == QN - 1 and pair == NPAIR - 1
                    emit_drain_pair(
                        qn, pair, ctx_ps, den_tiles[qn], scalar_den=last
                    )
                    del ctx_tiles[(qn, pair)]
                    d97 = den_tiles[qn]
                    slackq.append(
                        (
                            lambda qn=qn, p=pair, d=d97: emit_norm_rcp_pair(
                                qn, p, d
                            ),
                            0,
                        )
                    )
                    slackq.append(
                        (lambda qn=qn, p=pair: emit_norm_mul_pair(qn, p), 213)
                    )
                    if pair == NPAIR - 1:
                        for i2 in range(4):
                            enq_outproj(qn, i2)
                pops(nkc - (pair * (nkc // 2) + kc2 // 2))
            # endgame HAM keep-warm: the final norm chain leaves the PE
            # briefly idle, which would downshift the clock to 4/8 duty for
            # the whole output projection; dependency-free dummy matmuls
            # bridge the gap (ps_s banks are free after the last exp)
            for wi in range(15):
                wps = ps_s.tile([P, 2 * QW], F32, tag="s", name=f"ewarm{wi}")
                nc.tensor.matmul(
                    wps[:, 0:QW],
                    lhsT=warm_sb[:, 0:P],
                    rhs=warm_sb,
                    start=True,
                    stop=True,
                )
            while projq or slackq:
                pop_one()

    _split_multi_waits(nc)
    _PROGRAM = nc
    return nc


def _make_mask2():
    # mask2[:, 512j:512j+512] = stair(j): [k, q] = 1.0 iff q >= 128j + k
    k = np.arange(P)[:, None]
    q = np.arange(QW)[None, :]
    blocks = [(q >= 128 * j + k).astype(np.float32) for j in range(4)]
    return np.concatenate(blocks, axis=1)


def make_in_maps(x, Wq, Wk, Wv, Wo):
    import ml_dtypes

    nd = ml_dtypes.bfloat16 if F32R == BF16 else np.float32
    x = np.asarray(x, dtype=np.float32)
    mask2 = _make_mask2().astype(nd)
    # selection matrix for the softmax-normalization broadcast matmul:
    # bc[m] = rcp[0] for m<64 (he0), rcp[32] for m>=64 (he1)
    sel = np.zeros((33, P), np.float32)
    sel[0, 0:DK] = 1.0
    sel[32, DK:P] = 1.0
    sel = sel.astype(nd)
    Wq, Wk, Wv, Wo = (np.asarray(w, dtype=np.float32) for w in (Wq, Wk, Wv, Wo))
    # blocked x^T: [p, rc, f, c] so device rc-slices are contiguous
    xts = [
        np.ascontiguousarray(
            x[b].reshape(QN, QW, FC, P).transpose(3, 0, 2, 1).reshape(P, -1)
        ).astype(nd)
        for b in range(B)
    ]
    def qk_block(W, cols):
        # [p][pair][f][c]: per-pair slices are contiguous 2KB/partition
        s = W[:, cols].reshape(FC, P, NPAIR, P)
        return np.ascontiguousarray(
            s.transpose(1, 2, 0, 3).reshape(P, -1)
        ).astype(nd)

    def v_block(W, cols):
        # [p][f][c]: one contiguous 4KB run per partition
        s = W[:, cols].reshape(FC, P, CW)
        return np.ascontiguousarray(s.transpose(1, 0, 2).reshape(P, -1)).astype(nd)

    in_maps = []
    for c in range(NCORES):
        b, q4 = divmod(c, NCORES // B)
        cols = slice(q4 * CW, (q4 + 1) * CW)
        in_maps.append(
            {
                "xt": xts[b],
                "wq": qk_block(Wq, cols),
                "wk": qk_block(Wk, cols),
                "wv": v_block(Wv, cols),
                "wo": np.ascontiguousarray(Wo[cols, :]).astype(nd),
                "mask2": mask2,
                "sel": sel,
            }
        )
    return in_maps


def reduce_outputs(results):
    """Sum the per-core bf16 partials (4 cores per batch) in f64."""
    out = np.zeros((B, T, D), dtype=np.float64)
    for c in range(NCORES):
        b = c // (NCORES // B)
        out[b] += np.asarray(results[c]["out"], dtype=np.float64)
    return out.astype(np.float32)


def kernel(x, Wq, Wk, Wv, Wo):
    from concourse.bass_utils import run_bass_kernel_spmd

    nc = build_program()
    in_maps = make_in_maps(x, Wq, Wk, Wv, Wo)
    res = run_bass_kernel_spmd(nc, in_maps, core_ids=list(range(NCORES)))
    return reduce_outputs(res.results)


if __name__ == "__main__":
    rng = np.random.default_rng(0)
    s = 1.0 / np.sqrt(D)
    ins = {
        "x": rng.standard_normal((B, T, D)).astype(np.float32),
        "Wq": (rng.standard_normal((D, D)) * s).astype(np.float32),
        "Wk": (rng.standard_normal((D, D)) * s).astype(np.float32),
        "Wv": (rng.standard_normal((D, D)) * s).astype(np.float32),
        "Wo": (rng.standard_normal((D, D)) * (1.0 / np.sqrt(D))).astype(np.float32),
    }
    out = kernel(**ins)
    print("out", out.shape, out.dtype, float(np.abs(out).max()))

